# revision 9
# baseline (speedup 1.0000x reference)
"""Distributed Trainium2 Bass kernel for nn_Attention_57243324121446.

GQA attention (8 query groups, 1 kv head) with a pairwise-bias branch
(BatchRMSNorm -> exact gelu -> head projection, 4x nearest-neighbor upsample),
softclamp tanh, softmax, out-projection.

Sharding (8 cores): core c -> batch b = c//4, query groups {2*(c%4), 2*(c%4)+1}.
k/v are computed redundantly per core (single shared kv head). Pairwise is
sharded by (b, coarse-j block of 128 rows).

v2 layout (optimized):
 - Phase C (qkv+LN) overlaps phase B1 (pairwise stats streaming); the first
   16 of 32 pairwise tiles are cached in SBUF so B2 re-reads only half.
 - Bias exchange is an 8-rank AllToAll with duplicated head-pair chunks
   (wire ~0.9MB/half vs 3.7MB for the old 8-rank AllGather).
 - Attention is computed S^T = K^T q per j-chunk; P^T tiles feed AV matmuls
   as the *stationary* operand so the output lands as o[i, dv] with the
   softmax denominator accumulated for free in column 192 (ones column of v).
 - o is normalized per-i-partition (cheap [128,8] reciprocal), transposed via
   the PE into o^T, and AllGather'ed within the 4-core batch group per
   1024-token i-span; the out-projection for i-span 0 runs while span 1 is
   still computing.
"""

import os
import sys

sys.path.insert(0, "/opt/trn_rl_repo")

import numpy as np
import ml_dtypes

import concourse.bass as bass
import concourse.mybir as mybir
import concourse.tile as tile
from concourse.masks import make_identity


# --- workaround: this container's walrus caps CTRL instructions at 2 sem
# waits; Tile's kernel-tail drain can carry many. Split them across drains.
def _patched_drain_and_barrier(self, tick_clock, wait_clock):
    from concourse.vector_clock import ScopedClock
    drain_inst = self.nc.sync.drain()
    wait_clock.add_sem_waits(
        drain_inst.ins, ScopedClock({None: tick_clock.global_clock})
    )
    si = drain_inst.ins.sync_info
    if si is not None and len(si.on_wait) > 1:
        waits = list(si.on_wait)
        drain_inst.ins.sync_info = mybir.SyncInfo(
            on_wait=waits[:1], on_update=list(si.on_update)
        )
        for i in range(1, len(waits)):
            extra = self.nc.sync.drain()
            extra.ins.sync_info = mybir.SyncInfo(
                on_wait=waits[i:i + 1], on_update=[]
            )
    self.nc.all_engine_barrier()
    assert self.sems is not None
    popped = self.nc._tile_sem_poison_stack.pop()
    assert popped is self._sem_poison
    self.nc.clear_and_free_semaphores(list(self.sems.allocated().values()))
    self.nc.all_engine_barrier()


tile.TileContext._drain_and_barrier = _patched_drain_and_barrier


# --- workaround 2: this walrus accepts at most ONE sem wait per instruction.
# Rewrite the BIR json before compile: hoist excess waits onto same-engine
# Nop carriers inserted immediately before the offending instruction.
import json as _json
import concourse.bass_utils as _bass_utils
import concourse.bass2jax as _bass2jax


def _split_bir_multiwaits(bir_json):
    d = _json.loads(bir_json)
    mods = d.get("modules") or [d]
    for m in mods:
        for fn in m.get("functions", []):
            for bb in fn.get("blocks", []):
                out = []
                changed = False
                for ins in bb["instructions"]:
                    si = ins.get("sync_info")
                    w = (si or {}).get("on_wait") or []
                    if len(w) > 1 and ins.get("engine"):
                        eng = ins["engine"]
                        for i, wi in enumerate(w[:-1]):
                            out.append({
                                "debug": ins.get("debug"),
                                "engine": eng,
                                "ins": [{"dtype": "int32", "kind": "imm_value",
                                         "value": 0}],
                                "name": ins["name"] + f".sw{i}",
                                "opcode": "RegisterMove",
                                "outs": [{"dtype": "int32",
                                          "kind": "register_access",
                                          "regref": f"{eng}_zero"}],
                                "sync_info": {"on_update": [], "on_wait": [wi]},
                            })
                        si["on_wait"] = [w[-1]]
                        changed = True
                    out.append(ins)
                if changed:
                    bb["instructions"] = out
    return _json.dumps(d).encode()


_orig_compile_bir = _bass_utils.compile_bir_kernel


def _patched_compile_bir(bir_json, tmpdir, neff_name="file.neff"):
    return _orig_compile_bir(_split_bir_multiwaits(bir_json), tmpdir, neff_name)


_bass_utils.compile_bir_kernel = _patched_compile_bir
_bass2jax.compile_bir_kernel = _patched_compile_bir


# --- workaround 3: the agent image's antenv lacks axon_hooks, so the boot
# shim never registers the NTFF profile hook. Provide the module and install
# the ctypes hook ourselves so run_bass_kernel_spmd(trace=True) works.
def _install_ntff_hook():
    import types as _types
    mod = sys.modules.get("antenv.axon_hooks")
    if mod is None:
        mod = _types.ModuleType("antenv.axon_hooks")
        mod._hook = None
        def _set(h):
            mod._hook = h
        def _get():
            return mod._hook
        mod.set_axon_ntff_profile_hook = _set
        mod.get_axon_ntff_profile_hook = _get
        sys.modules["antenv.axon_hooks"] = mod
        import antenv as _antenv
        _antenv.axon_hooks = mod
    if mod._hook is None and os.path.exists("/opt/axon/libaxon_pjrt.so"):
        try:
            from trn_agent_boot.trn_boot import _ntff_profile_via_ctypes
            mod._hook = _ntff_profile_via_ctypes("/opt/axon/libaxon_pjrt.so")
        except Exception as e:
            print(f"ntff hook install failed: {e}", file=sys.stderr)


_install_ntff_hook()


BF16 = mybir.dt.bfloat16
FP16 = mybir.dt.float16
F32 = mybir.dt.float32
AF = mybir.ActivationFunctionType
ALU = mybir.AluOpType

B, N, D = 2, 2048, 1536
HEADS, KVH, DQK, DV = 8, 1, 128, 192
G = HEADS // KVH
NP, DP = 512, 128
SCALE = DQK ** -0.5
CLAMP = 5.0
MOMENTUM = 0.1
EPS = 1e-5

NCORES = 8
GPC = 2              # query groups per core
JBLK = NP // 4       # pairwise coarse-j rows per core = 128
ROWS = JBLK * NP     # pairwise rows per core = 65536
TOK = 128            # token chunk
NTOK = N // TOK      # 16
DCH = D // 128       # 12 d-model chunks
JC = N // 128        # 16 fine-j chunks
OUTC = D // 4        # 384 out cols per core
NPW = 32             # pairwise tiles of 2048 rows
KCACHE = 16          # pairwise tiles kept in SBUF between B1 and B2
MTOT = float(B * NP * NP)
HSP = N // 2         # i-span = 1024
J_ORDER = [j for j in range(JC) if j % 4 < 2] + [j for j in range(JC) if j % 4 >= 2]
RG8 = [list(range(NCORES))]
RG4 = [[0, 1, 2, 3], [4, 5, 6, 7]]


def _ap(base, dims):
    return bass.AP(tensor=base.tensor, offset=base.offset, ap=dims)


def build_graph():
    nc = bass.Bass()

    x_T = nc.declare_dram_parameter("x_T", [128, DCH, N], BF16, isOutput=False)
    pw_T = nc.declare_dram_parameter("pw_T", [128, ROWS], BF16, isOutput=False)
    w_qkv_c = nc.declare_dram_parameter("w_qkv_c", [128, DCH, 576], BF16, isOutput=False)
    w_bias_e = nc.declare_dram_parameter("w_bias_e", [128, 8], BF16, isOutput=False)
    w_out_c = nc.declare_dram_parameter("w_out_c", [128, DCH, OUTC], BF16, isOutput=False)
    b_out_c = nc.declare_dram_parameter("b_out_c", [1, OUTC], F32, isOutput=False)
    # vecs rows: 0 qw_eff,1 qb_eff,2 kw,3 kb,4 vw(192),5 vb(192),6 gamma,
    #            7 beta,8 rv9eps
    vecs = nc.declare_dram_parameter("vecs", [12, 192], F32, isOutput=False)
    bidx = nc.declare_dram_parameter("bidx", [GPC, JC, 128], mybir.dt.int32, isOutput=False)
    out_c = nc.declare_dram_parameter("out_c", [N, OUTC], F32, isOutput=True)

    with tile.TileContext(nc) as tc:
        with tc.tile_pool(name="const", bufs=1) as const, \
             tc.tile_pool(name="dram", bufs=1, space="DRAM") as dram:

            # ---------------- constants ----------------
            ident = const.tile([128, 128], BF16)
            make_identity(nc, ident[:])
            vec_sb = const.tile([128, 12], F32)
            nc.sync.dma_start(out=vec_sb[:], in_=_ap(vecs[:], [[1, 128], [192, 12]]))
            # vwb plane 0 = v_w broadcast, plane 1 = v_b broadcast (free dim)
            ones1 = const.tile([1, 128], BF16)
            nc.vector.memset(ones1[:], 1.0)
            # vwb_sb: col0 vw[0:128], col1 vb[0:128], col2 vw[128:192], col3 vb
            vwb_sb = const.tile([128, 4], F32)
            nc.sync.dma_start(
                out=vwb_sb[:, 0:2], in_=_ap(vecs[4, 0], [[1, 128], [192, 2]])
            )
            nc.sync.dma_start(
                out=vwb_sb[0:64, 2:4], in_=_ap(vecs[4, 128], [[1, 64], [192, 2]])
            )
            eps_sb = const.tile([128, 1], F32)
            nc.vector.memset(eps_sb[:], EPS)
            wq_sb = const.tile([128, DCH, 576], BF16)
            nc.sync.dma_start(out=wq_sb[:], in_=w_qkv_c[:])
            wb_sb = const.tile([128, 8], BF16)
            nc.sync.dma_start(out=wb_sb[:], in_=w_bias_e[:])
            bidx_sb = const.tile([128, GPC * JC], mybir.dt.int32)
            nc.sync.dma_start(
                out=bidx_sb[:], in_=_ap(bidx[:], [[1, 128], [128, GPC * JC]])
            )

            qkT = const.tile([128, 3, N], BF16)       # q0^T, q1^T, k^T
            vfull = const.tile([128, NTOK, 208], BF16)  # v~*vw cols 0:192, 192=1
            nc.vector.memset(vfull[:], 0.0)
            nc.vector.memset(vfull[:, :, 192:193], 1.0)
            stats = const.tile([128, NPW, 4, 6], F32)
            mv = const.tile([128, 2], F32)
            part = const.tile([128, 2], F32)
            gsum = const.tile([128, 2], F32)
            scl = const.tile([128, 2], F32)
            wo_sb = const.tile([128, DCH, OUTC], BF16)
            bout_bc = const.tile([128, OUTC], F32)

            var_in = dram.tile([128, 2], F32)
            var_out = dram.tile([128, 2], F32, addr_space="Shared")
            a2a_in_a = dram.tile([1024, 512], BF16)
            a2a_in_b = dram.tile([1024, 512], BF16)
            a2a_out_a = dram.tile([1024, 512], BF16)
            a2a_out_b = dram.tile([1024, 512], BF16)
            ot_own = [dram.tile([2 * DV, HSP], BF16, name=f"oto{i}") for i in range(2)]
            ot_all = [dram.tile([8 * DV, HSP], BF16, name=f"ota{i}") for i in range(2)]

            # ---------------- PE warmup (HAM unthrottle) ----------------
            with tc.tile_pool(name="warm", bufs=1) as warm, \
                 tc.tile_pool(name="warmps", bufs=1, space="PSUM") as warmps:
                wps = warmps.tile([128, 512], F32)
                wsb = warm.tile([128, 128], F32)
                wdr = dram.tile([128, 128], F32)
                for r in range(100):
                    nc.tensor.matmul(
                        wps[:, 0:128], ident[:], ident[:],
                        start=True, stop=True,
                    )
                nc.vector.tensor_copy(wsb[:, 0:128], wps[:, 0:128])
                nc.sync.dma_start(out=wdr[:], in_=wsb[:])

            # ---------------- B1 (pairwise stats) overlapped with C ----------------
            with tc.tile_pool(name="cpool", bufs=1) as cpool, \
                 tc.tile_pool(name="cps", bufs=2, space="PSUM") as cps:
                pwc = cpool.tile([128, KCACHE, 2048], BF16, name="pwc")
                for t in range(NTOK):
                    xt_t = cpool.tile([128, DCH, TOK], BF16, tag="xt", bufs=3)
                    nc.sync.dma_start(out=xt_t[:], in_=x_T[:, :, t * TOK:(t + 1) * TOK])
                    nc.sync.dma_start(
                        out=pwc[:, t, :], in_=pw_T[:, t * 2048:(t + 1) * 2048]
                    )
                    ps_qkv = cps.tile([128, 576], F32, tag="qkv")
                    for c in range(DCH):
                        nc.tensor.matmul(
                            ps_qkv[:, 0:512], xt_t[:, c, :],
                            wq_sb[:, c, 0:512], start=(c == 0),
                            stop=(c == DCH - 1),
                        )
                        nc.tensor.matmul(
                            ps_qkv[:, 512:576], xt_t[:, c, :],
                            wq_sb[:, c, 512:576], start=(c == 0),
                            stop=(c == DCH - 1),
                        )
                    qkv_t = cpool.tile([128, 576], BF16, tag="qkvt", bufs=3)
                    nc.scalar.copy(qkv_t[:], ps_qkv[:])
                    cst = cpool.tile([128, 4, 6], F32, tag="cst", bufs=2)
                    cmv = cpool.tile([128, 4, 2], F32, tag="cmv", bufs=2)
                    for s in range(4):
                        lo, hi = (s * 128, (s + 1) * 128) if s < 3 else (384, 576)
                        nc.vector.bn_stats(
                            out=cst[:, s, :], in_=qkv_t[:, lo:hi]
                        )
                        nc.vector.bn_aggr(out=cmv[:, s, :], in_=cst[:, s, :])
                    cmvf = cmv[:].rearrange("p s d -> p (s d)")
                    sd = cpool.tile([128, 4], F32, tag="sd", bufs=2)
                    nc.scalar.activation(
                        out=sd[:], in_=cmvf[:, 1::2], func=AF.Sqrt, bias=eps_sb[:]
                    )
                    nc.vector.reciprocal(out=sd[:], in_=sd[:])
                    nmu = cpool.tile([128, 4], F32, tag="nmu", bufs=2)
                    nc.vector.tensor_tensor(
                        out=nmu[:], in0=cmvf[:, 0::2], in1=sd[:], op=ALU.mult
                    )
                    nc.vector.tensor_scalar_mul(nmu[:], nmu[:], -1.0)
                    nrm = cpool.tile([128, 576], BF16, tag="nrm", bufs=3)
                    for s in range(4):
                        lo, hi = (s * 128, (s + 1) * 128) if s < 3 else (384, 576)
                        nc.vector.tensor_scalar(
                            out=nrm[:, lo:hi], in0=qkv_t[:, lo:hi],
                            scalar1=sd[:, s:s + 1], scalar2=nmu[:, s:s + 1],
                            op0=ALU.mult, op1=ALU.add,
                        )
                    nc.vector.tensor_copy(vfull[:, t, 0:192], nrm[:, 384:576])
                    for s in range(3):
                        ps_tr = cps.tile([128, 128], BF16, tag="tr")
                        nc.tensor.transpose(
                            ps_tr[:], nrm[:, s * 128:(s + 1) * 128], ident[:]
                        )
                        av = 0 if s < 2 else 2
                        nc.scalar.activation(
                            out=qkT[:, s, t * TOK:(t + 1) * TOK], in_=ps_tr[:],
                            func=AF.Identity, scale=vec_sb[:, av:av + 1],
                            bias=vec_sb[:, av + 1:av + 2],
                        )
                    # B1 stats on the cached pairwise tile
                    for q2 in range(4):
                        nc.vector.bn_stats(
                            out=stats[:, t, q2, :],
                            in_=pwc[:, t, q2 * 512:(q2 + 1) * 512],
                        )
                # uncached pairwise tiles: stream for stats only
                for u in range(KCACHE, NPW):
                    ptl = cpool.tile([128, 2048], BF16, tag="pwtail", bufs=3)
                    nc.sync.dma_start(
                        out=ptl[:], in_=pw_T[:, u * 2048:(u + 1) * 2048]
                    )
                    for q2 in range(4):
                        nc.vector.bn_stats(
                            out=stats[:, u, q2, :],
                            in_=ptl[:, q2 * 512:(q2 + 1) * 512],
                        )
                nc.vector.bn_aggr(
                    out=mv[:], in_=stats[:].rearrange("p a b c -> p (a b) c")
                )
                nc.vector.tensor_scalar_mul(part[:, 0:1], mv[:, 0:1], float(ROWS))
                nc.vector.tensor_tensor(
                    out=part[:, 1:2], in0=mv[:, 0:1], in1=mv[:, 0:1], op=ALU.mult
                )
                nc.vector.tensor_tensor(
                    out=part[:, 1:2], in0=part[:, 1:2], in1=mv[:, 1:2], op=ALU.add
                )
                nc.vector.tensor_scalar_mul(part[:, 1:2], part[:, 1:2], float(ROWS))
                nc.sync.dma_start(out=var_in[:], in_=part[:])
                nc.gpsimd.collective_compute(
                    "AllReduce", ALU.add,
                    replica_groups=RG8,
                    ins=[var_in[:].opt()], outs=[var_out[:].opt()],
                )
                nc.sync.dma_start(out=gsum[:], in_=var_out[:])

                # ---------------- B-scale ----------------
                nc.vector.tensor_scalar_mul(scl[:, 0:1], gsum[:, 0:1], 1.0 / MTOT)
                nc.vector.tensor_tensor(
                    out=scl[:, 0:1], in0=scl[:, 0:1], in1=scl[:, 0:1], op=ALU.mult
                )
                nc.vector.tensor_scalar_mul(scl[:, 1:2], gsum[:, 1:2], 1.0 / MTOT)
                nc.vector.tensor_tensor(
                    out=scl[:, 0:1], in0=scl[:, 1:2], in1=scl[:, 0:1],
                    op=ALU.subtract
                )
                nc.vector.tensor_scalar(
                    out=scl[:, 0:1], in0=scl[:, 0:1], scalar1=MOMENTUM,
                    scalar2=vec_sb[:, 8:9], op0=ALU.mult, op1=ALU.add,
                )
                nc.scalar.activation(out=scl[:, 0:1], in_=scl[:, 0:1], func=AF.Sqrt)
                nc.vector.reciprocal(out=scl[:, 0:1], in_=scl[:, 0:1])
                nc.vector.tensor_tensor(
                    out=scl[:, 0:1], in0=scl[:, 0:1], in1=vec_sb[:, 6:7],
                    op=ALU.mult
                )
                nc.vector.tensor_copy(scl[:, 1:2], vec_sb[:, 7:8])

                # ---------------- B2: gelu + bias projection ----------------
                # A2A in-buffer rows: dup*512 + h*64 + jl_half (jl_half < 64)
                with tc.tile_pool(name="b2", bufs=1) as b2, \
                     tc.tile_pool(name="b2ps", bufs=2, space="PSUM") as b2ps:
                    for hg in range(4):
                        acc = b2.tile([128, 8, 512], BF16, tag="acc", bufs=2)
                        for tl in range(8):
                            t = hg * 8 + tl
                            if t < KCACHE:
                                src = pwc[:, t, :]
                            else:
                                pt2 = b2.tile([128, 2048], BF16, tag="pw2", bufs=3)
                                nc.sync.dma_start(
                                    out=pt2[:],
                                    in_=pw_T[:, t * 2048:(t + 1) * 2048],
                                )
                                src = pt2[:]
                            gel = b2.tile([128, 2048], BF16, tag="gel", bufs=3)
                            nc.scalar.activation(
                                out=gel[:], in_=src, func=AF.Gelu,
                                bias=scl[:, 1:2], scale=scl[:, 0:1],
                            )
                            ps_b = b2ps.tile([128, 512], F32, tag="psb")
                            # full-tile init: the copy below reads all 128
                            # partitions but only rows 32q..32q+8 are written
                            # by the matmuls
                            nc.vector.memset(ps_b[:], 0.0)
                            for q in range(4):
                                nc.tensor.matmul(
                                    ps_b[32 * q:32 * q + 8, :], wb_sb[:],
                                    gel[:, q * 512:(q + 1) * 512],
                                    start=True, stop=True,
                                    tile_position=(0, 32 * q),
                                )
                            nc.vector.tensor_copy(acc[:, tl, :], ps_b[:])
                        dst = a2a_in_a if hg < 2 else a2a_in_b
                        for q in range(4):
                            for dup in range(2):
                                nc.sync.dma_start(
                                    out=_ap(
                                        dst[dup * 512 + (hg % 2) * 32 + q, 0],
                                        [[64 * 512, 8], [4 * 512, 8], [1, 512]],
                                    ),
                                    in_=acc[32 * q:32 * q + 8, :, :],
                                )
                        if hg == 1:
                            nc.gpsimd.collective_compute(
                                "AllToAll", ALU.bypass,
                                replica_groups=RG8,
                                ins=[a2a_in_a[:].opt()],
                                outs=[a2a_out_a[:].opt()],
                            )
                    nc.gpsimd.collective_compute(
                        "AllToAll", ALU.bypass,
                        replica_groups=RG8,
                        ins=[a2a_in_b[:].opt()], outs=[a2a_out_b[:].opt()],
                    )

            # load E-phase constants (issued late so they don't delay pw/x)
            nc.sync.dma_start(out=wo_sb[:], in_=w_out_c[:])
            nc.sync.dma_start(
                out=bout_bc[:], in_=_ap(b_out_c[:], [[0, 128], [1, OUTC]])
            )

            # ---------------- D: attention, E: out projection ----------------
            with tc.tile_pool(name="dper", bufs=1) as dper, \
                 tc.tile_pool(name="dsb", bufs=3) as dsb, \
                 tc.tile_pool(name="osb", bufs=2) as osb, \
                 tc.tile_pool(name="dps", bufs=2, space="PSUM") as dps, \
                 tc.tile_pool(name="dpo", bufs=1, space="PSUM") as dpo:
                bias_gath = dper.tile([128, GPC, JC, 512], BF16)
                for j in J_ORDER:
                    for g in range(GPC):
                        src_t = a2a_out_a if (j % 4) < 2 else a2a_out_b
                        nc.gpsimd.indirect_dma_start(
                            out=bias_gath[:, g, j, :],
                            out_offset=None,
                            in_=src_t[:],
                            in_offset=bass.IndirectOffsetOnAxis(
                                ap=bidx_sb[:, g * JC + j:g * JC + j + 1], axis=0
                            ),
                        )

                def d_pass(isp, g):
                    i0 = isp * HSP
                    ps_oa = dpo.tile([128, HSP], F32, tag="oa", name="ps_oa")
                    ps_ob = dpo.tile([65, HSP], F32, tag="ob", name="ps_ob")
                    for jj, j in enumerate(J_ORDER):
                        ps_s = dps.tile([128, 1024], F32, tag="s", name="ps_s")
                        nc.tensor.matmul(
                            ps_s[:, 0:512],
                            qkT[:, 2, j * 128:(j + 1) * 128],
                            qkT[:, g, i0:i0 + 512],
                            start=True, stop=True,
                        )
                        nc.tensor.matmul(
                            ps_s[:, 512:1024],
                            qkT[:, 2, j * 128:(j + 1) * 128],
                            qkT[:, g, i0 + 512:i0 + 1024],
                            start=True, stop=True,
                        )
                        bt = bias_gath[:, g, j, i0 // 4:i0 // 4 + 256]
                        bb = _ap(bt, [bt.ap[0], bt.ap[1], [0, 4]])
                        u = dsb.tile([128, 1024], FP16, tag="u", name="u")
                        nc.vector.tensor_tensor(
                            out=u[:].rearrange("p (a b) -> p a b", b=4),
                            in0=ps_s[:].rearrange("p (a b) -> p a b", b=4),
                            in1=bb, op=ALU.add,
                        )
                        ut = dsb.tile([128, 1024], FP16, tag="ut", name="ut")
                        nc.scalar.activation(out=ut[:], in_=u[:], func=AF.Tanh)
                        pT = dsb.tile([128, 1024], BF16, tag="pT", name="pT")
                        nc.scalar.activation(
                            out=pT[:], in_=ut[:], func=AF.Exp, scale=CLAMP
                        )
                        for h2 in range(2):
                            nc.tensor.matmul(
                                ps_oa[:, h2 * 512:(h2 + 1) * 512],
                                vfull[:, j, 0:128],
                                pT[:, h2 * 512:(h2 + 1) * 512],
                                start=(jj == 0), stop=(jj == JC - 1),
                            )
                        for h2 in range(2):
                            nc.tensor.matmul(
                                ps_ob[:, h2 * 512:(h2 + 1) * 512],
                                vfull[:, j, 128:193],
                                pT[:, h2 * 512:(h2 + 1) * 512],
                                start=(jj == 0), stop=(jj == JC - 1),
                            )
                    rd = osb.tile([1, HSP], F32, tag="rd", name="rd")
                    nc.vector.reciprocal(rd[:], ps_ob[64:65, :])
                    rdb = osb.tile([1, HSP], BF16, tag="rdb", name="rdb")
                    nc.vector.tensor_copy(rdb[:], rd[:])
                    ps_bc = dps.tile([128, HSP], F32, tag="s", name="ps_bc")
                    for h2 in range(2):
                        nc.tensor.matmul(
                            ps_bc[:, h2 * 512:(h2 + 1) * 512], ones1[:],
                            rdb[:, h2 * 512:(h2 + 1) * 512],
                            start=True, stop=True,
                        )
                    bc_sb = osb.tile([128, HSP], BF16, tag="bc_sb", name="bc_sb")
                    nc.vector.tensor_copy(bc_sb[:], ps_bc[:])
                    oa = osb.tile([128, HSP], BF16, tag="oa_sb", name="oa")
                    ob = osb.tile([64, HSP], BF16, tag="ob_sb", name="ob")
                    nc.vector.tensor_tensor(
                        out=oa[:], in0=ps_oa[:], in1=bc_sb[:], op=ALU.mult
                    )
                    nc.vector.tensor_scalar(
                        out=oa[:], in0=oa[:], scalar1=vwb_sb[:, 0:1],
                        scalar2=vwb_sb[:, 1:2], op0=ALU.mult, op1=ALU.add,
                    )
                    nc.vector.tensor_tensor(
                        out=ob[:], in0=ps_ob[0:64, :], in1=bc_sb[0:64, :],
                        op=ALU.mult,
                    )
                    nc.vector.tensor_scalar(
                        out=ob[:], in0=ob[:], scalar1=vwb_sb[0:64, 2:3],
                        scalar2=vwb_sb[0:64, 3:4], op0=ALU.mult, op1=ALU.add,
                    )
                    nc.sync.dma_start(
                        out=ot_own[isp][g * DV:g * DV + 128, :], in_=oa[:]
                    )
                    nc.sync.dma_start(
                        out=ot_own[isp][g * DV + 128:g * DV + DV, :], in_=ob[:]
                    )

                def ag_pass(isp):
                    nc.gpsimd.collective_compute(
                        "AllGather", ALU.bypass,
                        replica_groups=RG4,
                        ins=[ot_own[isp][:].opt()], outs=[ot_all[isp][:].opt()],
                    )

                def e_pass(isp):
                    mge = dper.tile([128, DCH, HSP], BF16, name=f"mge{isp}")
                    nc.sync.dma_start(
                        out=mge[:],
                        in_=_ap(ot_all[isp][0, 0],
                                [[HSP, 128], [128 * HSP, DCH], [1, HSP]]),
                    )
                    for tl in range(8):
                        t = isp * 8 + tl
                        ps_o = dpo.tile([128, OUTC], F32, tag="oa", name="ps_o")
                        for kc in range(DCH):
                            nc.tensor.matmul(
                                ps_o[:], mge[:, kc, tl * 128:(tl + 1) * 128],
                                wo_sb[:, kc, :], start=(kc == 0),
                                stop=(kc == DCH - 1),
                            )
                        o_out = osb.tile([128, OUTC], F32, tag="oout", name="o_out")
                        nc.vector.tensor_tensor(
                            out=o_out[:], in0=ps_o[:], in1=bout_bc[:], op=ALU.add
                        )
                        nc.sync.dma_start(
                            out=out_c[t * TOK:(t + 1) * TOK, :], in_=o_out[:]
                        )

                d_pass(0, 0)
                d_pass(0, 1)
                ag_pass(0)
                d_pass(1, 0)
                e_pass(0)
                d_pass(1, 1)
                ag_pass(1)
                e_pass(1)

    return nc


def prepare_in_maps(inputs):
    bf16 = ml_dtypes.bfloat16
    x = np.asarray(inputs["x"], np.float32)
    pairwise = np.asarray(inputs["pairwise"], np.float32)
    w_qkv = np.asarray(inputs["w_qkv"], np.float32)
    q_w = np.asarray(inputs["q_w"], np.float32)
    q_b = np.asarray(inputs["q_b"], np.float32)
    k_w = np.asarray(inputs["k_w"], np.float32)
    k_b = np.asarray(inputs["k_b"], np.float32)
    v_w = np.asarray(inputs["v_w"], np.float32)
    v_b = np.asarray(inputs["v_b"], np.float32)
    gamma = np.asarray(inputs["bias_gamma"], np.float32)
    beta = np.asarray(inputs["bias_beta"], np.float32)
    rvar = np.asarray(inputs["bias_running_var"], np.float32)
    w_bias = np.asarray(inputs["w_bias"], np.float32)
    w_out = np.asarray(inputs["w_out"], np.float32)
    b_out = np.asarray(inputs["b_out"], np.float32)

    vecs = np.zeros((12, 192), np.float32)
    vecs[0, :128] = q_w * (SCALE / CLAMP)
    vecs[1, :128] = q_b * (SCALE / CLAMP)
    vecs[2, :128] = k_w
    vecs[3, :128] = k_b
    vecs[4, :192] = v_w
    vecs[5, :192] = v_b
    vecs[6, :128] = gamma
    vecs[7, :128] = beta
    vecs[8, :128] = (1.0 - MOMENTUM) * rvar + EPS

    w_bias_e = (w_bias / CLAMP).astype(bf16)

    in_maps = []
    for c in range(NCORES):
        b, a = divmod(c, 4)
        xt = np.ascontiguousarray(
            x[b].T.reshape(DCH, 128, N).transpose(1, 0, 2)
        ).astype(bf16)
        pw = pairwise[b, :, a * JBLK:(a + 1) * JBLK, :]        # [i, jl, dp]
        pw = np.ascontiguousarray(pw.transpose(2, 1, 0).reshape(128, ROWS)
                                  ).astype(bf16)
        qcols = w_qkv[:, 2 * a * DQK:(2 * a + 2) * DQK]
        kcols = w_qkv[:, G * DQK:G * DQK + DQK]
        vcols = w_qkv[:, G * DQK + DQK:]
        wq = np.concatenate([qcols, kcols, vcols], axis=1)     # [1536, 576]
        wq = np.ascontiguousarray(
            wq.reshape(DCH, 128, 576).transpose(1, 0, 2)).astype(bf16)
        wo = w_out[:, a * OUTC:(a + 1) * OUTC]
        wo = np.ascontiguousarray(
            wo.reshape(DCH, 128, OUTC).transpose(1, 0, 2)).astype(bf16)
        # bias gather rows in the A2A out half-buffer [1024, 512]:
        # row = (b*4 + block)*128 + g*64 + jl_half, jl_half = ((j%4)%2)*32 + p//4
        gg, jj, pp = np.meshgrid(
            np.arange(GPC), np.arange(JC), np.arange(128), indexing="ij"
        )
        bidx_np = (
            (b * 4 + jj // 4) * 128 + gg * 64 + ((jj % 4) % 2) * 32 + pp // 4
        ).astype(np.int32)
        in_maps.append({
            "x_T": xt,
            "pw_T": pw,
            "w_qkv_c": wq,
            "w_bias_e": w_bias_e,
            "w_out_c": wo,
            "b_out_c": b_out[None, a * OUTC:(a + 1) * OUTC].astype(np.float32),
            "vecs": vecs,
            "bidx": bidx_np,
        })
    return in_maps


_NC_CACHE = None


def _get_nc():
    global _NC_CACHE
    if _NC_CACHE is None:
        _NC_CACHE = build_graph()
    return _NC_CACHE


def kernel(**inputs):
    from concourse.bass_utils import run_bass_kernel_spmd

    in_maps = prepare_in_maps(inputs)
    nc = _get_nc()
    res = run_bass_kernel_spmd(
        nc, in_maps, core_ids=list(range(NCORES)),
        trace=bool(int(os.environ.get("BASS_KERNEL_TRACE", "0"))),
        tmpdir=os.environ.get("BASS_KERNEL_TMPDIR"),
    )
    if res.exec_time_ns is not None:
        print(f"HW exec time: {res.exec_time_ns} ns", file=sys.stderr)

    out = np.zeros((B, N, D), np.float32)
    for c in range(NCORES):
        b, a = divmod(c, 4)
        out[b, :, a * OUTC:(a + 1) * OUTC] = res.results[c]["out_c"]
    return out


# revision 12
# speedup vs baseline: 1.0383x; 1.0383x over previous
"""Distributed Trainium2 Bass kernel for nn_Attention_57243324121446.

GQA attention (8 query groups, 1 kv head) with a pairwise-bias branch
(BatchRMSNorm -> exact gelu -> head projection, 4x nearest-neighbor upsample),
softclamp tanh, softmax, out-projection.

Sharding (8 cores): core c -> batch b = c//4, query groups {2*(c%4), 2*(c%4)+1}.
k/v are computed redundantly per core (single shared kv head). Pairwise is
sharded by (b, coarse-j block of 128 rows).

v2 layout (optimized):
 - Phase C (qkv+LN) overlaps phase B1 (pairwise stats streaming); the first
   16 of 32 pairwise tiles are cached in SBUF so B2 re-reads only half.
 - Bias exchange is an 8-rank AllToAll with duplicated head-pair chunks
   (wire ~0.9MB/half vs 3.7MB for the old 8-rank AllGather).
 - Attention is computed S^T = K^T q per j-chunk; P^T tiles feed AV matmuls
   as the *stationary* operand so the output lands as o[i, dv] with the
   softmax denominator accumulated for free in column 192 (ones column of v).
 - o is normalized per-i-partition (cheap [128,8] reciprocal), transposed via
   the PE into o^T, and AllGather'ed within the 4-core batch group per
   1024-token i-span; the out-projection for i-span 0 runs while span 1 is
   still computing.
"""

import os
import sys

sys.path.insert(0, "/opt/trn_rl_repo")

import numpy as np
import ml_dtypes

import concourse.bass as bass
import concourse.mybir as mybir
import concourse.tile as tile
from concourse.masks import make_identity


# --- workaround: this container's walrus caps CTRL instructions at 2 sem
# waits; Tile's kernel-tail drain can carry many. Split them across drains.
def _patched_drain_and_barrier(self, tick_clock, wait_clock):
    from concourse.vector_clock import ScopedClock
    drain_inst = self.nc.sync.drain()
    wait_clock.add_sem_waits(
        drain_inst.ins, ScopedClock({None: tick_clock.global_clock})
    )
    si = drain_inst.ins.sync_info
    if si is not None and len(si.on_wait) > 1:
        waits = list(si.on_wait)
        drain_inst.ins.sync_info = mybir.SyncInfo(
            on_wait=waits[:1], on_update=list(si.on_update)
        )
        for i in range(1, len(waits)):
            extra = self.nc.sync.drain()
            extra.ins.sync_info = mybir.SyncInfo(
                on_wait=waits[i:i + 1], on_update=[]
            )
    self.nc.all_engine_barrier()
    assert self.sems is not None
    popped = self.nc._tile_sem_poison_stack.pop()
    assert popped is self._sem_poison
    self.nc.clear_and_free_semaphores(list(self.sems.allocated().values()))
    self.nc.all_engine_barrier()


tile.TileContext._drain_and_barrier = _patched_drain_and_barrier


# --- workaround 2: this walrus accepts at most ONE sem wait per instruction.
# Rewrite the BIR json before compile: hoist excess waits onto same-engine
# Nop carriers inserted immediately before the offending instruction.
import json as _json
import concourse.bass_utils as _bass_utils
import concourse.bass2jax as _bass2jax


def _split_bir_multiwaits(bir_json):
    d = _json.loads(bir_json)
    mods = d.get("modules") or [d]
    for m in mods:
        for fn in m.get("functions", []):
            for bb in fn.get("blocks", []):
                out = []
                changed = False
                for ins in bb["instructions"]:
                    si = ins.get("sync_info")
                    w = (si or {}).get("on_wait") or []
                    if len(w) > 1 and ins.get("engine"):
                        eng = ins["engine"]
                        for i, wi in enumerate(w[:-1]):
                            out.append({
                                "debug": ins.get("debug"),
                                "engine": eng,
                                "ins": [{"dtype": "int32", "kind": "imm_value",
                                         "value": 0}],
                                "name": ins["name"] + f".sw{i}",
                                "opcode": "RegisterMove",
                                "outs": [{"dtype": "int32",
                                          "kind": "register_access",
                                          "regref": f"{eng}_zero"}],
                                "sync_info": {"on_update": [], "on_wait": [wi]},
                            })
                        si["on_wait"] = [w[-1]]
                        changed = True
                    out.append(ins)
                if changed:
                    bb["instructions"] = out
    return _json.dumps(d).encode()


_orig_compile_bir = _bass_utils.compile_bir_kernel


def _patched_compile_bir(bir_json, tmpdir, neff_name="file.neff"):
    return _orig_compile_bir(_split_bir_multiwaits(bir_json), tmpdir, neff_name)


_bass_utils.compile_bir_kernel = _patched_compile_bir
_bass2jax.compile_bir_kernel = _patched_compile_bir


# --- workaround 3: the agent image's antenv lacks axon_hooks, so the boot
# shim never registers the NTFF profile hook. Provide the module and install
# the ctypes hook ourselves so run_bass_kernel_spmd(trace=True) works.
def _install_ntff_hook():
    import types as _types
    mod = sys.modules.get("antenv.axon_hooks")
    if mod is None:
        mod = _types.ModuleType("antenv.axon_hooks")
        mod._hook = None
        def _set(h):
            mod._hook = h
        def _get():
            return mod._hook
        mod.set_axon_ntff_profile_hook = _set
        mod.get_axon_ntff_profile_hook = _get
        sys.modules["antenv.axon_hooks"] = mod
        import antenv as _antenv
        _antenv.axon_hooks = mod
    if mod._hook is None and os.path.exists("/opt/axon/libaxon_pjrt.so"):
        try:
            from trn_agent_boot.trn_boot import _ntff_profile_via_ctypes
            mod._hook = _ntff_profile_via_ctypes("/opt/axon/libaxon_pjrt.so")
        except Exception as e:
            print(f"ntff hook install failed: {e}", file=sys.stderr)


_install_ntff_hook()


BF16 = mybir.dt.bfloat16
FP16 = mybir.dt.float16
F32 = mybir.dt.float32
AF = mybir.ActivationFunctionType
ALU = mybir.AluOpType

B, N, D = 2, 2048, 1536
HEADS, KVH, DQK, DV = 8, 1, 128, 192
G = HEADS // KVH
NP, DP = 512, 128
SCALE = DQK ** -0.5
CLAMP = 5.0
MOMENTUM = 0.1
EPS = 1e-5

NCORES = 8
GPC = 2              # query groups per core
JBLK = NP // 4       # pairwise coarse-j rows per core = 128
ROWS = JBLK * NP     # pairwise rows per core = 65536
TOK = 128            # token chunk
NTOK = N // TOK      # 16
DCH = D // 128       # 12 d-model chunks
JC = N // 128        # 16 fine-j chunks
OUTC = D // 4        # 384 out cols per core
NPW = 32             # pairwise tiles of 2048 rows
KCACHE = 16          # pairwise tiles kept in SBUF between B1 and B2
MTOT = float(B * NP * NP)
HSP = N // 2         # i-span = 1024
J_ORDER = [j for j in range(JC) if j % 4 < 2] + [j for j in range(JC) if j % 4 >= 2]
RG8 = [list(range(NCORES))]
RG4 = [[0, 1, 2, 3], [4, 5, 6, 7]]


def _ap(base, dims):
    return bass.AP(tensor=base.tensor, offset=base.offset, ap=dims)


def build_graph():
    nc = bass.Bass()

    x_T = nc.declare_dram_parameter("x_T", [128, DCH, N], BF16, isOutput=False)
    pw_T = nc.declare_dram_parameter("pw_T", [128, ROWS], BF16, isOutput=False)
    w_qkv_c = nc.declare_dram_parameter("w_qkv_c", [128, DCH, 576], BF16, isOutput=False)
    w_bias_e = nc.declare_dram_parameter("w_bias_e", [128, 8], BF16, isOutput=False)
    w_out_c = nc.declare_dram_parameter("w_out_c", [128, DCH, OUTC], BF16, isOutput=False)
    b_out_c = nc.declare_dram_parameter("b_out_c", [1, OUTC], F32, isOutput=False)
    # vecs rows: 0 qw_eff,1 qb_eff,2 kw,3 kb,4 vw(192),5 vb(192),6 gamma,
    #            7 beta,8 rv9eps
    vecs = nc.declare_dram_parameter("vecs", [12, 192], F32, isOutput=False)
    bidx = nc.declare_dram_parameter("bidx", [GPC, JC, 128], mybir.dt.int32, isOutput=False)
    out_c = nc.declare_dram_parameter("out_c", [N, OUTC], F32, isOutput=True)

    with tile.TileContext(nc) as tc:
        with tc.tile_pool(name="const", bufs=1) as const, \
             tc.tile_pool(name="dram", bufs=1, space="DRAM") as dram:

            # ---------------- constants ----------------
            ident = const.tile([128, 128], BF16)
            make_identity(nc, ident[:])
            vec_sb = const.tile([128, 12], F32)
            nc.sync.dma_start(out=vec_sb[:], in_=_ap(vecs[:], [[1, 128], [192, 12]]))
            # vwb plane 0 = v_w broadcast, plane 1 = v_b broadcast (free dim)
            ones1 = const.tile([1, 128], BF16)
            nc.vector.memset(ones1[:], 1.0)
            # vwb_sb: col0 vw[0:128], col1 vb[0:128], col2 vw[128:192], col3 vb
            vwb_sb = const.tile([128, 4], F32)
            nc.sync.dma_start(
                out=vwb_sb[:, 0:2], in_=_ap(vecs[4, 0], [[1, 128], [192, 2]])
            )
            nc.sync.dma_start(
                out=vwb_sb[0:64, 2:4], in_=_ap(vecs[4, 128], [[1, 64], [192, 2]])
            )
            eps_sb = const.tile([128, 1], F32)
            nc.vector.memset(eps_sb[:], EPS)
            wq_sb = const.tile([128, DCH, 576], BF16)
            nc.sync.dma_start(out=wq_sb[:], in_=w_qkv_c[:])
            wb_sb = const.tile([128, 8], BF16)
            nc.sync.dma_start(out=wb_sb[:], in_=w_bias_e[:])
            bidx_sb = const.tile([128, GPC * JC], mybir.dt.int32)
            nc.sync.dma_start(
                out=bidx_sb[:], in_=_ap(bidx[:], [[1, 128], [128, GPC * JC]])
            )

            bias_gath = const.tile([128, GPC, JC, 512], BF16)
            qkT = const.tile([128, 3, N], BF16)       # q0^T, q1^T, k^T
            vfull = const.tile([128, NTOK, 208], BF16)  # v~*vw cols 0:192, 192=1
            nc.vector.memset(vfull[:], 0.0)
            nc.vector.memset(vfull[:, :, 192:193], 1.0)
            stats = const.tile([128, 8, 16, 6], F32)
            mv = const.tile([128, 2], F32)
            part = const.tile([128, 2], F32)
            gsum = const.tile([128, 2], F32)
            scl = const.tile([128, 2], F32)
            wo_sb = const.tile([128, DCH, OUTC], BF16)
            bout_bc = const.tile([128, OUTC], F32)

            var_in = dram.tile([128, 2], F32)
            var_out = dram.tile([128, 2], F32, addr_space="Shared")
            a2a_in_a = dram.tile([1024, 512], BF16)
            a2a_in_b = dram.tile([1024, 512], BF16)
            a2a_out_a = dram.tile([1024, 512], BF16)
            a2a_out_b = dram.tile([1024, 512], BF16)
            SPANS = [(0, 1024), (1024, 512), (1536, 512)]
            ot_own = [dram.tile([2 * DV, ln], BF16, name=f"oto{i}")
                      for i, (o, ln) in enumerate(SPANS)]
            ot_all = [dram.tile([8 * DV, ln], BF16, name=f"ota{i}")
                      for i, (o, ln) in enumerate(SPANS)]

            # ---------------- PE warmup (HAM unthrottle) ----------------
            with tc.tile_pool(name="warm", bufs=1) as warm, \
                 tc.tile_pool(name="warmps", bufs=1, space="PSUM") as warmps:
                wps = warmps.tile([128, 512], F32)
                wsb = warm.tile([128, 128], F32)
                wdr = dram.tile([128, 128], F32)
                for r in range(100):
                    nc.tensor.matmul(
                        wps[:, 0:128], ident[:], ident[:],
                        start=True, stop=True,
                    )
                nc.vector.tensor_copy(wsb[:, 0:128], wps[:, 0:128])
                nc.sync.dma_start(out=wdr[:], in_=wsb[:])

            # ---------------- B1 (pairwise stats) + C (qkv/LN), overlapped ----------------
            with tc.tile_pool(name="cpool", bufs=1) as cpool, \
                 tc.tile_pool(name="cps", bufs=2, space="PSUM") as cps:
                xt_sb = cpool.tile([128, DCH, N], BF16, name="xt_sb")
                nc.sync.dma_start(out=xt_sb[:], in_=x_T[:])
                # B1 first so the DVE queue drains stats (and fires the
                # AllReduce) before C's LN work
                for tb in range(8):
                    ptl = cpool.tile([128, 4, 2048], BF16, tag="pw", bufs=2)
                    nc.sync.dma_start(
                        out=ptl[:], in_=pw_T[:, tb * 8192:(tb + 1) * 8192]
                    )
                    for q2 in range(16):
                        nc.vector.bn_stats(
                            out=stats[:, tb, q2, :],
                            in_=ptl[:, q2 // 4, (q2 % 4) * 512:(q2 % 4 + 1) * 512],
                        )
                nc.vector.bn_aggr(
                    out=mv[:], in_=stats[:].rearrange("p a b c -> p (a b) c")
                )
                nc.vector.tensor_scalar_mul(part[:, 0:1], mv[:, 0:1], float(ROWS))
                nc.vector.tensor_tensor(
                    out=part[:, 1:2], in0=mv[:, 0:1], in1=mv[:, 0:1], op=ALU.mult
                )
                nc.vector.tensor_tensor(
                    out=part[:, 1:2], in0=part[:, 1:2], in1=mv[:, 1:2], op=ALU.add
                )
                nc.vector.tensor_scalar_mul(part[:, 1:2], part[:, 1:2], float(ROWS))
                nc.sync.dma_start(out=var_in[:], in_=part[:])
                nc.gpsimd.collective_compute(
                    "AllReduce", ALU.add,
                    replica_groups=RG8,
                    ins=[var_in[:].opt()], outs=[var_out[:].opt()],
                )
                nc.sync.dma_start(out=gsum[:], in_=var_out[:])

                # C: qkv projection + per-range LN + transposes
                for t in range(NTOK):
                    ps_qkv = cps.tile([128, 576], F32, tag="qkv")
                    for c in range(DCH):
                        nc.tensor.matmul(
                            ps_qkv[:, 0:512], xt_sb[:, c, t * TOK:(t + 1) * TOK],
                            wq_sb[:, c, 0:512], start=(c == 0),
                            stop=(c == DCH - 1),
                        )
                        nc.tensor.matmul(
                            ps_qkv[:, 512:576], xt_sb[:, c, t * TOK:(t + 1) * TOK],
                            wq_sb[:, c, 512:576], start=(c == 0),
                            stop=(c == DCH - 1),
                        )
                    qkv_t = cpool.tile([128, 576], BF16, tag="qkvt", bufs=3)
                    nc.scalar.copy(qkv_t[:], ps_qkv[:])
                    cst = cpool.tile([128, 4, 6], F32, tag="cst", bufs=2)
                    cmv = cpool.tile([128, 4, 2], F32, tag="cmv", bufs=2)
                    for s in range(4):
                        lo, hi = (s * 128, (s + 1) * 128) if s < 3 else (384, 576)
                        nc.vector.bn_stats(
                            out=cst[:, s, :], in_=qkv_t[:, lo:hi]
                        )
                        nc.vector.bn_aggr(out=cmv[:, s, :], in_=cst[:, s, :])
                    cmvf = cmv[:].rearrange("p s d -> p (s d)")
                    sd = cpool.tile([128, 4], F32, tag="sd", bufs=2)
                    nc.scalar.activation(
                        out=sd[:], in_=cmvf[:, 1::2], func=AF.Sqrt, bias=eps_sb[:]
                    )
                    nc.vector.reciprocal(out=sd[:], in_=sd[:])
                    nmu = cpool.tile([128, 4], F32, tag="nmu", bufs=2)
                    nc.vector.tensor_tensor(
                        out=nmu[:], in0=cmvf[:, 0::2], in1=sd[:], op=ALU.mult
                    )
                    nc.vector.tensor_scalar_mul(nmu[:], nmu[:], -1.0)
                    nrm = cpool.tile([128, 576], BF16, tag="nrm", bufs=3)
                    for s in range(4):
                        lo, hi = (s * 128, (s + 1) * 128) if s < 3 else (384, 576)
                        nc.vector.tensor_scalar(
                            out=nrm[:, lo:hi], in0=qkv_t[:, lo:hi],
                            scalar1=sd[:, s:s + 1], scalar2=nmu[:, s:s + 1],
                            op0=ALU.mult, op1=ALU.add,
                        )
                    nc.vector.tensor_copy(vfull[:, t, 0:192], nrm[:, 384:576])
                    for s in range(3):
                        ps_tr = cps.tile([128, 128], BF16, tag="tr")
                        nc.tensor.transpose(
                            ps_tr[:], nrm[:, s * 128:(s + 1) * 128], ident[:]
                        )
                        av = 0 if s < 2 else 2
                        nc.scalar.activation(
                            out=qkT[:, s, t * TOK:(t + 1) * TOK], in_=ps_tr[:],
                            func=AF.Identity, scale=vec_sb[:, av:av + 1],
                            bias=vec_sb[:, av + 1:av + 2],
                        )

                # ---------------- B-scale ----------------
                nc.vector.tensor_scalar_mul(scl[:, 0:1], gsum[:, 0:1], 1.0 / MTOT)
                nc.vector.tensor_tensor(
                    out=scl[:, 0:1], in0=scl[:, 0:1], in1=scl[:, 0:1], op=ALU.mult
                )
                nc.vector.tensor_scalar_mul(scl[:, 1:2], gsum[:, 1:2], 1.0 / MTOT)
                nc.vector.tensor_tensor(
                    out=scl[:, 0:1], in0=scl[:, 1:2], in1=scl[:, 0:1],
                    op=ALU.subtract
                )
                nc.vector.tensor_scalar(
                    out=scl[:, 0:1], in0=scl[:, 0:1], scalar1=MOMENTUM,
                    scalar2=vec_sb[:, 8:9], op0=ALU.mult, op1=ALU.add,
                )
                nc.scalar.activation(out=scl[:, 0:1], in_=scl[:, 0:1], func=AF.Sqrt)
                nc.vector.reciprocal(out=scl[:, 0:1], in_=scl[:, 0:1])
                nc.vector.tensor_tensor(
                    out=scl[:, 0:1], in0=scl[:, 0:1], in1=vec_sb[:, 6:7],
                    op=ALU.mult
                )
                nc.vector.tensor_copy(scl[:, 1:2], vec_sb[:, 7:8])

            # ---------------- B2: gelu + bias projection ----------------
            # A2A in-buffer rows: dup*512 + h*64 + jl_half (jl_half < 64)
            with tc.tile_pool(name="b2", bufs=1) as b2, \
                 tc.tile_pool(name="b2ps", bufs=2, space="PSUM") as b2ps:
                for hb in range(8):
                    pt2 = b2.tile([128, 4, 2048], BF16, tag="pw2", bufs=2)
                    nc.sync.dma_start(
                        out=pt2[:], in_=pw_T[:, hb * 8192:(hb + 1) * 8192]
                    )
                    acc = b2.tile([128, 4, 512], BF16, tag="acc", bufs=2)
                    for tl in range(4):
                        gel = b2.tile([128, 2048], BF16, tag="gel", bufs=3)
                        nc.scalar.activation(
                            out=gel[:], in_=pt2[:, tl, :], func=AF.Gelu,
                            bias=scl[:, 1:2], scale=scl[:, 0:1],
                        )
                        ps_b = b2ps.tile([128, 512], F32, tag="psb")
                        # full-tile init: the copy below reads all 128
                        # partitions but only rows 32q..32q+8 are written
                        nc.vector.memset(ps_b[:], 0.0)
                        for q in range(4):
                            nc.tensor.matmul(
                                ps_b[32 * q:32 * q + 8, :], wb_sb[:],
                                gel[:, q * 512:(q + 1) * 512],
                                start=True, stop=True,
                                tile_position=(0, 32 * q),
                            )
                        nc.vector.tensor_copy(acc[:, tl, :], ps_b[:])
                    dst = a2a_in_a if hb < 4 else a2a_in_b
                    for q in range(4):
                        for dup in range(2):
                            nc.sync.dma_start(
                                out=_ap(
                                    dst[dup * 512 + (hb % 4) * 16 + q, 0],
                                    [[64 * 512, 8], [4 * 512, 4], [1, 512]],
                                ),
                                in_=acc[32 * q:32 * q + 8, :, :],
                            )
                    if hb == 3:
                        nc.gpsimd.collective_compute(
                            "AllToAll", ALU.bypass,
                            replica_groups=RG8,
                            ins=[a2a_in_a[:].opt()],
                            outs=[a2a_out_a[:].opt()],
                        )
                nc.gpsimd.collective_compute(
                    "AllToAll", ALU.bypass,
                    replica_groups=RG8,
                    ins=[a2a_in_b[:].opt()], outs=[a2a_out_b[:].opt()],
                )

            # load E-phase constants (issued late so they don't delay pw/x)
            nc.sync.dma_start(out=wo_sb[:], in_=w_out_c[:])
            nc.sync.dma_start(
                out=bout_bc[:], in_=_ap(b_out_c[:], [[0, 128], [1, OUTC]])
            )

            # ---------------- D: attention, E: out projection ----------------
            with tc.tile_pool(name="dper", bufs=1) as dper, \
                 tc.tile_pool(name="dsb", bufs=3) as dsb, \
                 tc.tile_pool(name="osb", bufs=2) as osb, \
                 tc.tile_pool(name="dps", bufs=2, space="PSUM") as dps, \
                 tc.tile_pool(name="dpo", bufs=1, space="PSUM") as dpo:
                for j in J_ORDER:
                    for g in range(GPC):
                        src_t = a2a_out_a if (j % 4) < 2 else a2a_out_b
                        nc.gpsimd.indirect_dma_start(
                            out=bias_gath[:, g, j, :],
                            out_offset=None,
                            in_=src_t[:],
                            in_offset=bass.IndirectOffsetOnAxis(
                                ap=bidx_sb[:, g * JC + j:g * JC + j + 1], axis=0
                            ),
                        )

                def d_pass(sp, g):
                    i0, ln = SPANS[sp]
                    nh = ln // 512
                    ps_oa = dpo.tile([128, 1024], F32, tag="oa", name="ps_oa")
                    ps_ob = dpo.tile([65, 1024], F32, tag="ob", name="ps_ob")
                    for jj, j in enumerate(J_ORDER):
                        ps_s = dps.tile([128, 1024], F32, tag="s", name="ps_s")
                        for h2 in range(nh):
                            nc.tensor.matmul(
                                ps_s[:, h2 * 512:(h2 + 1) * 512],
                                qkT[:, 2, j * 128:(j + 1) * 128],
                                qkT[:, g, i0 + h2 * 512:i0 + (h2 + 1) * 512],
                                start=True, stop=True,
                            )
                        bt = bias_gath[:, g, j, i0 // 4:i0 // 4 + ln // 4]
                        bb = _ap(bt, [bt.ap[0], bt.ap[1], [0, 4]])
                        u = dsb.tile([128, 1024], FP16, tag="u", name="u")
                        nc.vector.tensor_tensor(
                            out=u[:, 0:ln].rearrange("p (a b) -> p a b", b=4),
                            in0=ps_s[:, 0:ln].rearrange("p (a b) -> p a b", b=4),
                            in1=bb, op=ALU.add,
                        )
                        ut = dsb.tile([128, 1024], FP16, tag="ut", name="ut")
                        nc.scalar.activation(
                            out=ut[:, 0:ln], in_=u[:, 0:ln], func=AF.Tanh
                        )
                        pT = dsb.tile([128, 1024], BF16, tag="pT", name="pT")
                        nc.scalar.activation(
                            out=pT[:, 0:ln], in_=ut[:, 0:ln], func=AF.Exp,
                            scale=CLAMP
                        )
                        for h2 in range(nh):
                            nc.tensor.matmul(
                                ps_oa[:, h2 * 512:(h2 + 1) * 512],
                                vfull[:, j, 0:128],
                                pT[:, h2 * 512:(h2 + 1) * 512],
                                start=(jj == 0), stop=(jj == JC - 1),
                            )
                        for h2 in range(nh):
                            nc.tensor.matmul(
                                ps_ob[:, h2 * 512:(h2 + 1) * 512],
                                vfull[:, j, 128:193],
                                pT[:, h2 * 512:(h2 + 1) * 512],
                                start=(jj == 0), stop=(jj == JC - 1),
                            )
                    rd = osb.tile([1, 1024], F32, tag="rd", name="rd")
                    nc.vector.reciprocal(rd[:, 0:ln], ps_ob[64:65, 0:ln])
                    rdb = osb.tile([1, 1024], BF16, tag="rdb", name="rdb")
                    nc.vector.tensor_copy(rdb[:, 0:ln], rd[:, 0:ln])
                    ps_bc = dps.tile([128, 1024], F32, tag="s", name="ps_bc")
                    for h2 in range(nh):
                        nc.tensor.matmul(
                            ps_bc[:, h2 * 512:(h2 + 1) * 512], ones1[:],
                            rdb[:, h2 * 512:(h2 + 1) * 512],
                            start=True, stop=True,
                        )
                    bc_sb = osb.tile([128, 1024], BF16, tag="bc_sb", name="bc_sb")
                    nc.vector.tensor_copy(bc_sb[:, 0:ln], ps_bc[:, 0:ln])
                    oa = osb.tile([128, 1024], BF16, tag="oa_sb", name="oa")
                    ob = osb.tile([64, 1024], BF16, tag="ob_sb", name="ob")
                    nc.vector.scalar_tensor_tensor(
                        out=oa[:, 0:ln], in0=bc_sb[:, 0:ln],
                        scalar=vwb_sb[:, 0:1], in1=ps_oa[:, 0:ln],
                        op0=ALU.mult, op1=ALU.mult,
                    )
                    nc.vector.tensor_scalar(
                        out=oa[:, 0:ln], in0=oa[:, 0:ln], scalar1=1.0,
                        scalar2=vwb_sb[:, 1:2], op0=ALU.mult, op1=ALU.add,
                    )
                    nc.vector.scalar_tensor_tensor(
                        out=ob[:, 0:ln], in0=bc_sb[0:64, 0:ln],
                        scalar=vwb_sb[0:64, 2:3], in1=ps_ob[0:64, 0:ln],
                        op0=ALU.mult, op1=ALU.mult,
                    )
                    nc.vector.tensor_scalar(
                        out=ob[:, 0:ln], in0=ob[:, 0:ln], scalar1=1.0,
                        scalar2=vwb_sb[0:64, 3:4], op0=ALU.mult, op1=ALU.add,
                    )
                    nc.sync.dma_start(
                        out=ot_own[sp][g * DV:g * DV + 128, :], in_=oa[:, 0:ln]
                    )
                    nc.sync.dma_start(
                        out=ot_own[sp][g * DV + 128:g * DV + DV, :],
                        in_=ob[:, 0:ln]
                    )

                def ag_pass(sp):
                    nc.gpsimd.collective_compute(
                        "AllGather", ALU.bypass,
                        replica_groups=RG4,
                        ins=[ot_own[sp][:].opt()], outs=[ot_all[sp][:].opt()],
                    )

                def e_pass(sp):
                    i0, ln = SPANS[sp]
                    mge = dper.tile([128, DCH, ln], BF16, name=f"mge{sp}")
                    nc.sync.dma_start(
                        out=mge[:],
                        in_=_ap(ot_all[sp][0, 0],
                                [[ln, 128], [128 * ln, DCH], [1, ln]]),
                    )
                    for tl in range(ln // TOK):
                        t = i0 // TOK + tl
                        ps_o = dpo.tile(
                            [128, OUTC], F32, tag=("oa" if tl % 2 == 0 else "ob"),
                            name="ps_o",
                        )
                        for kc in range(DCH):
                            nc.tensor.matmul(
                                ps_o[:], mge[:, kc, tl * 128:(tl + 1) * 128],
                                wo_sb[:, kc, :], start=(kc == 0),
                                stop=(kc == DCH - 1),
                            )
                        o_out = osb.tile([128, OUTC], F32, tag="oout", name="o_out")
                        nc.vector.tensor_tensor(
                            out=o_out[:], in0=ps_o[:], in1=bout_bc[:], op=ALU.add
                        )
                        nc.sync.dma_start(
                            out=out_c[t * TOK:(t + 1) * TOK, :], in_=o_out[:]
                        )

                d_pass(0, 0)
                d_pass(0, 1)
                ag_pass(0)
                d_pass(1, 0)
                d_pass(1, 1)
                ag_pass(1)
                d_pass(2, 0)
                e_pass(0)
                d_pass(2, 1)
                ag_pass(2)
                e_pass(1)
                e_pass(2)

    return nc


def prepare_in_maps(inputs):
    bf16 = ml_dtypes.bfloat16
    x = np.asarray(inputs["x"], np.float32)
    pairwise = np.asarray(inputs["pairwise"], np.float32)
    w_qkv = np.asarray(inputs["w_qkv"], np.float32)
    q_w = np.asarray(inputs["q_w"], np.float32)
    q_b = np.asarray(inputs["q_b"], np.float32)
    k_w = np.asarray(inputs["k_w"], np.float32)
    k_b = np.asarray(inputs["k_b"], np.float32)
    v_w = np.asarray(inputs["v_w"], np.float32)
    v_b = np.asarray(inputs["v_b"], np.float32)
    gamma = np.asarray(inputs["bias_gamma"], np.float32)
    beta = np.asarray(inputs["bias_beta"], np.float32)
    rvar = np.asarray(inputs["bias_running_var"], np.float32)
    w_bias = np.asarray(inputs["w_bias"], np.float32)
    w_out = np.asarray(inputs["w_out"], np.float32)
    b_out = np.asarray(inputs["b_out"], np.float32)

    vecs = np.zeros((12, 192), np.float32)
    vecs[0, :128] = q_w * (SCALE / CLAMP)
    vecs[1, :128] = q_b * (SCALE / CLAMP)
    vecs[2, :128] = k_w
    vecs[3, :128] = k_b
    vecs[4, :192] = v_w
    vecs[5, :192] = v_b
    vecs[6, :128] = gamma
    vecs[7, :128] = beta
    vecs[8, :128] = (1.0 - MOMENTUM) * rvar + EPS

    w_bias_e = (w_bias / CLAMP).astype(bf16)

    in_maps = []
    for c in range(NCORES):
        b, a = divmod(c, 4)
        xt = np.ascontiguousarray(
            x[b].T.reshape(DCH, 128, N).transpose(1, 0, 2)
        ).astype(bf16)
        pw = pairwise[b, :, a * JBLK:(a + 1) * JBLK, :]        # [i, jl, dp]
        pw = np.ascontiguousarray(pw.transpose(2, 1, 0).reshape(128, ROWS)
                                  ).astype(bf16)
        qcols = w_qkv[:, 2 * a * DQK:(2 * a + 2) * DQK]
        kcols = w_qkv[:, G * DQK:G * DQK + DQK]
        vcols = w_qkv[:, G * DQK + DQK:]
        wq = np.concatenate([qcols, kcols, vcols], axis=1)     # [1536, 576]
        wq = np.ascontiguousarray(
            wq.reshape(DCH, 128, 576).transpose(1, 0, 2)).astype(bf16)
        wo = w_out[:, a * OUTC:(a + 1) * OUTC]
        wo = np.ascontiguousarray(
            wo.reshape(DCH, 128, OUTC).transpose(1, 0, 2)).astype(bf16)
        # bias gather rows in the A2A out half-buffer [1024, 512]:
        # row = (b*4 + block)*128 + g*64 + jl_half, jl_half = ((j%4)%2)*32 + p//4
        gg, jj, pp = np.meshgrid(
            np.arange(GPC), np.arange(JC), np.arange(128), indexing="ij"
        )
        bidx_np = (
            (b * 4 + jj // 4) * 128 + gg * 64 + ((jj % 4) % 2) * 32 + pp // 4
        ).astype(np.int32)
        in_maps.append({
            "x_T": xt,
            "pw_T": pw,
            "w_qkv_c": wq,
            "w_bias_e": w_bias_e,
            "w_out_c": wo,
            "b_out_c": b_out[None, a * OUTC:(a + 1) * OUTC].astype(np.float32),
            "vecs": vecs,
            "bidx": bidx_np,
        })
    return in_maps


_NC_CACHE = None


def _get_nc():
    global _NC_CACHE
    if _NC_CACHE is None:
        _NC_CACHE = build_graph()
    return _NC_CACHE


def kernel(**inputs):
    from concourse.bass_utils import run_bass_kernel_spmd

    in_maps = prepare_in_maps(inputs)
    nc = _get_nc()
    res = run_bass_kernel_spmd(
        nc, in_maps, core_ids=list(range(NCORES)),
        trace=bool(int(os.environ.get("BASS_KERNEL_TRACE", "0"))),
        tmpdir=os.environ.get("BASS_KERNEL_TMPDIR"),
    )
    if res.exec_time_ns is not None:
        print(f"HW exec time: {res.exec_time_ns} ns", file=sys.stderr)

    out = np.zeros((B, N, D), np.float32)
    for c in range(NCORES):
        b, a = divmod(c, 4)
        out[b, :, a * OUTC:(a + 1) * OUTC] = res.results[c]["out_c"]
    return out


# revision 18
# speedup vs baseline: 1.1622x; 1.1193x over previous
"""Distributed Trainium2 Bass kernel for nn_Attention_57243324121446.

GQA attention (8 query groups, 1 kv head) with a pairwise-bias branch
(BatchRMSNorm -> exact gelu -> head projection, 4x nearest-neighbor upsample),
softclamp tanh, softmax, out-projection.

Sharding (8 cores): core c -> batch b = c//4, query groups {2*(c%4), 2*(c%4)+1}.
k/v are computed redundantly per core (single shared kv head). Pairwise is
sharded by (b, coarse-j block of 128 rows).

v2 layout (optimized):
 - Phase C (qkv+LN) overlaps phase B1 (pairwise stats streaming); the first
   16 of 32 pairwise tiles are cached in SBUF so B2 re-reads only half.
 - Bias exchange is an 8-rank AllToAll with duplicated head-pair chunks
   (wire ~0.9MB/half vs 3.7MB for the old 8-rank AllGather).
 - Attention is computed S^T = K^T q per j-chunk; P^T tiles feed AV matmuls
   as the *stationary* operand so the output lands as o[i, dv] with the
   softmax denominator accumulated for free in column 192 (ones column of v).
 - o is normalized per-i-partition (cheap [128,8] reciprocal), transposed via
   the PE into o^T, and AllGather'ed within the 4-core batch group per
   1024-token i-span; the out-projection for i-span 0 runs while span 1 is
   still computing.
"""

import os
import sys

sys.path.insert(0, "/opt/trn_rl_repo")

import numpy as np
import ml_dtypes

import concourse.bass as bass
import concourse.mybir as mybir
import concourse.tile as tile
from concourse.masks import make_identity


# --- workaround: this container's walrus caps CTRL instructions at 2 sem
# waits; Tile's kernel-tail drain can carry many. Split them across drains.
def _patched_drain_and_barrier(self, tick_clock, wait_clock):
    from concourse.vector_clock import ScopedClock
    drain_inst = self.nc.sync.drain()
    wait_clock.add_sem_waits(
        drain_inst.ins, ScopedClock({None: tick_clock.global_clock})
    )
    si = drain_inst.ins.sync_info
    if si is not None and len(si.on_wait) > 1:
        waits = list(si.on_wait)
        drain_inst.ins.sync_info = mybir.SyncInfo(
            on_wait=waits[:1], on_update=list(si.on_update)
        )
        for i in range(1, len(waits)):
            extra = self.nc.sync.drain()
            extra.ins.sync_info = mybir.SyncInfo(
                on_wait=waits[i:i + 1], on_update=[]
            )
    self.nc.all_engine_barrier()
    assert self.sems is not None
    popped = self.nc._tile_sem_poison_stack.pop()
    assert popped is self._sem_poison
    self.nc.clear_and_free_semaphores(list(self.sems.allocated().values()))
    self.nc.all_engine_barrier()


tile.TileContext._drain_and_barrier = _patched_drain_and_barrier


# --- workaround 2: this walrus accepts at most ONE sem wait per instruction.
# Rewrite the BIR json before compile: hoist excess waits onto same-engine
# Nop carriers inserted immediately before the offending instruction.
import json as _json
import concourse.bass_utils as _bass_utils
import concourse.bass2jax as _bass2jax


def _split_bir_multiwaits(bir_json):
    d = _json.loads(bir_json)
    mods = d.get("modules") or [d]
    for m in mods:
        for fn in m.get("functions", []):
            for bb in fn.get("blocks", []):
                out = []
                changed = False
                for ins in bb["instructions"]:
                    si = ins.get("sync_info")
                    w = (si or {}).get("on_wait") or []
                    if len(w) > 1 and ins.get("engine"):
                        eng = ins["engine"]
                        for i, wi in enumerate(w[:-1]):
                            out.append({
                                "debug": ins.get("debug"),
                                "engine": eng,
                                "ins": [{"dtype": "int32", "kind": "imm_value",
                                         "value": 0}],
                                "name": ins["name"] + f".sw{i}",
                                "opcode": "RegisterMove",
                                "outs": [{"dtype": "int32",
                                          "kind": "register_access",
                                          "regref": f"{eng}_zero"}],
                                "sync_info": {"on_update": [], "on_wait": [wi]},
                            })
                        si["on_wait"] = [w[-1]]
                        changed = True
                    out.append(ins)
                if changed:
                    bb["instructions"] = out
    return _json.dumps(d).encode()


_orig_compile_bir = _bass_utils.compile_bir_kernel


def _patched_compile_bir(bir_json, tmpdir, neff_name="file.neff"):
    return _orig_compile_bir(_split_bir_multiwaits(bir_json), tmpdir, neff_name)


_bass_utils.compile_bir_kernel = _patched_compile_bir
_bass2jax.compile_bir_kernel = _patched_compile_bir


# --- workaround 3: the agent image's antenv lacks axon_hooks, so the boot
# shim never registers the NTFF profile hook. Provide the module and install
# the ctypes hook ourselves so run_bass_kernel_spmd(trace=True) works.
def _install_ntff_hook():
    import types as _types
    mod = sys.modules.get("antenv.axon_hooks")
    if mod is None:
        mod = _types.ModuleType("antenv.axon_hooks")
        mod._hook = None
        def _set(h):
            mod._hook = h
        def _get():
            return mod._hook
        mod.set_axon_ntff_profile_hook = _set
        mod.get_axon_ntff_profile_hook = _get
        sys.modules["antenv.axon_hooks"] = mod
        import antenv as _antenv
        _antenv.axon_hooks = mod
    if mod._hook is None and os.path.exists("/opt/axon/libaxon_pjrt.so"):
        try:
            from trn_agent_boot.trn_boot import _ntff_profile_via_ctypes
            mod._hook = _ntff_profile_via_ctypes("/opt/axon/libaxon_pjrt.so")
        except Exception as e:
            print(f"ntff hook install failed: {e}", file=sys.stderr)


_install_ntff_hook()


BF16 = mybir.dt.bfloat16
FP16 = mybir.dt.float16
F32 = mybir.dt.float32
AF = mybir.ActivationFunctionType
ALU = mybir.AluOpType

B, N, D = 2, 2048, 1536
HEADS, KVH, DQK, DV = 8, 1, 128, 192
G = HEADS // KVH
NP, DP = 512, 128
SCALE = DQK ** -0.5
CLAMP = 5.0
MOMENTUM = 0.1
EPS = 1e-5

NCORES = 8
GPC = 2              # query groups per core
JBLK = NP // 4       # pairwise coarse-j rows per core = 128
ROWS = JBLK * NP     # pairwise rows per core = 65536
TOK = 128            # token chunk
NTOK = N // TOK      # 16
DCH = D // 128       # 12 d-model chunks
JC = N // 128        # 16 fine-j chunks
OUTC = D // 4        # 384 out cols per core
NPW = 32             # pairwise tiles of 2048 rows
KCACHE = 16          # pairwise tiles kept in SBUF between B1 and B2
MTOT = float(B * NP * NP)
HSP = N // 2         # i-span = 1024
J_ORDER = [j for j in range(JC) if j % 4 < 2] + [j for j in range(JC) if j % 4 >= 2]
RG8 = [list(range(NCORES))]
RG4 = [[0, 1, 2, 3], [4, 5, 6, 7]]


def _ap(base, dims):
    return bass.AP(tensor=base.tensor, offset=base.offset, ap=dims)


def build_graph():
    nc = bass.Bass()

    x_T = nc.declare_dram_parameter("x_T", [128, DCH, N], BF16, isOutput=False)
    pw_T = nc.declare_dram_parameter("pw_T", [128, ROWS], BF16, isOutput=False)
    w_qkv_c = nc.declare_dram_parameter("w_qkv_c", [128, DCH, 576], BF16, isOutput=False)
    w_bias_e = nc.declare_dram_parameter("w_bias_e", [128, 8], BF16, isOutput=False)
    w_out_c = nc.declare_dram_parameter("w_out_c", [128, DCH, OUTC], BF16, isOutput=False)
    b_out_c = nc.declare_dram_parameter("b_out_c", [1, OUTC], F32, isOutput=False)
    # vecs rows: 0 qw_eff,1 qb_eff,2 kw,3 kb,4 vw(192),5 vb(192),6 gamma,
    #            7 beta,8 rv9eps
    vecs = nc.declare_dram_parameter("vecs", [12, 192], F32, isOutput=False)
    bidx = nc.declare_dram_parameter("bidx", [GPC, JC, 128], mybir.dt.int32, isOutput=False)
    out_c = nc.declare_dram_parameter("out_c", [N, OUTC], F32, isOutput=True)

    with tile.TileContext(nc) as tc:
        with tc.tile_pool(name="const", bufs=1) as const, \
             tc.tile_pool(name="dram", bufs=1, space="DRAM") as dram:

            # ---------------- constants ----------------
            ident = const.tile([128, 128], BF16)
            make_identity(nc, ident[:])
            vec_sb = const.tile([128, 12], F32)
            nc.sync.dma_start(out=vec_sb[:], in_=_ap(vecs[:], [[1, 128], [192, 12]]))
            # vwb plane 0 = v_w broadcast, plane 1 = v_b broadcast (free dim)
            ones1 = const.tile([1, 128], BF16)
            nc.vector.memset(ones1[:], 1.0)
            # vwb_sb: col0 vw[0:128], col1 vb[0:128], col2 vw[128:192], col3 vb
            vwb_sb = const.tile([128, 4], F32)
            nc.sync.dma_start(
                out=vwb_sb[:, 0:2], in_=_ap(vecs[4, 0], [[1, 128], [192, 2]])
            )
            nc.sync.dma_start(
                out=vwb_sb[0:64, 2:4], in_=_ap(vecs[4, 128], [[1, 64], [192, 2]])
            )
            eps_sb = const.tile([128, 1], F32)
            nc.vector.memset(eps_sb[:], EPS)
            wq_sb = const.tile([128, DCH, 576], BF16)
            nc.sync.dma_start(out=wq_sb[:], in_=w_qkv_c[:])
            wb_sb = const.tile([128, 8], BF16)
            nc.sync.dma_start(out=wb_sb[:], in_=w_bias_e[:])
            bidx_sb = const.tile([128, GPC * JC], mybir.dt.int32)
            nc.sync.dma_start(
                out=bidx_sb[:], in_=_ap(bidx[:], [[1, 128], [128, GPC * JC]])
            )

            bias_gath = const.tile([128, GPC, JC, 512], BF16)
            qkT = const.tile([128, 3, N], BF16)       # q0^T, q1^T, k^T
            vfull = const.tile([128, NTOK, 208], BF16)  # v~*vw cols 0:192, 192=1
            nc.vector.memset(vfull[:], 0.0)
            nc.vector.memset(vfull[:, :, 192:193], 1.0)
            stats = const.tile([128, 8, 16, 6], F32)
            mv = const.tile([128, 2], F32)
            part = const.tile([128, 2], F32)
            gsum = const.tile([128, 2], F32)
            scl = const.tile([128, 2], F32)
            wo_sb = const.tile([128, DCH, OUTC], BF16)
            bout_bc = const.tile([128, OUTC], F32)

            var_in = dram.tile([128, 2], F32)
            var_out = dram.tile([128, 2], F32, addr_space="Shared")
            a2a_in_a = dram.tile([1024, 512], BF16)
            a2a_in_b = dram.tile([1024, 512], BF16)
            a2a_out_a = dram.tile([1024, 512], BF16)
            a2a_out_b = dram.tile([1024, 512], BF16)
            SPANS = [(0, 1024), (1024, 512), (1536, 512)]
            ot_own = [dram.tile([2 * DV, ln], BF16, name=f"oto{i}")
                      for i, (o, ln) in enumerate(SPANS)]
            ot_all = [dram.tile([8 * DV, ln], BF16, name=f"ota{i}")
                      for i, (o, ln) in enumerate(SPANS)]

            # ---------------- PE warmup (HAM unthrottle) ----------------
            with tc.tile_pool(name="warm", bufs=1) as warm, \
                 tc.tile_pool(name="warmps", bufs=1, space="PSUM") as warmps:
                wps = warmps.tile([128, 512], F32)
                wsb = warm.tile([128, 128], F32)
                wdr = dram.tile([128, 128], F32)
                for r in range(100):
                    nc.tensor.matmul(
                        wps[:, 0:128], ident[:], ident[:],
                        start=True, stop=True,
                    )
                nc.vector.tensor_copy(wsb[:, 0:128], wps[:, 0:128])
                nc.sync.dma_start(out=wdr[:], in_=wsb[:])

            # ---------------- B1 (pairwise stats) + C (qkv/LN), overlapped ----------------
            with tc.tile_pool(name="cpool", bufs=1) as cpool, \
                 tc.tile_pool(name="cps", bufs=2, space="PSUM") as cps:
                xt_sb = cpool.tile([128, DCH, N], BF16, name="xt_sb")
                nc.sync.dma_start(out=xt_sb[:], in_=x_T[:])
                qkv_sb = cpool.tile([128, NTOK, 576], BF16, name="qkv_sb")
                st_all = cpool.tile([128, NTOK, 4, 6], F32, name="st_all")
                mv_all = cpool.tile([128, NTOK, 4, 2], F32, name="mv_all")
                std_all = cpool.tile([128, NTOK * 4], F32, name="std_all")
                pwt = [None] * 8
                for tb in range(8):
                    pwt[tb] = cpool.tile([128, 4, 2048], BF16, tag="pw", bufs=3,
                                         name="pwt")
                    nc.sync.dma_start(
                        out=pwt[tb][:], in_=pw_T[:, tb * 8192:(tb + 1) * 8192]
                    )
                for t in range(NTOK):
                    ps_qkv = cps.tile([128, 576], F32, tag="qkv")
                    for c in range(DCH):
                        nc.tensor.matmul(
                            ps_qkv[:, 0:512], xt_sb[:, c, t * TOK:(t + 1) * TOK],
                            wq_sb[:, c, 0:512], start=(c == 0),
                            stop=(c == DCH - 1),
                        )
                        nc.tensor.matmul(
                            ps_qkv[:, 512:576], xt_sb[:, c, t * TOK:(t + 1) * TOK],
                            wq_sb[:, c, 512:576], start=(c == 0),
                            stop=(c == DCH - 1),
                        )
                    nc.scalar.copy(qkv_sb[:, t, :], ps_qkv[:])
                    for sr in range(4):
                        lo, hi = (sr * 128, (sr + 1) * 128) if sr < 3 else (384, 576)
                        nc.vector.bn_stats(
                            out=st_all[:, t, sr, :], in_=qkv_sb[:, t, lo:hi]
                        )
                        nc.vector.bn_aggr(
                            out=mv_all[:, t, sr, :], in_=st_all[:, t, sr, :]
                        )
                    # interleave B1 stats: 8 per C tile (pw big-tile t//2)
                    tb, hq = t // 2, (t % 2) * 8
                    for q2 in range(hq, hq + 8):
                        nc.vector.bn_stats(
                            out=stats[:, tb, q2, :],
                            in_=pwt[tb][:, q2 // 4, (q2 % 4) * 512:(q2 % 4 + 1) * 512],
                        )
                # aggregate + AllReduce as soon as stats are done
                nc.vector.bn_aggr(
                    out=mv[:], in_=stats[:].rearrange("p a b c -> p (a b) c")
                )
                nc.vector.tensor_scalar_mul(part[:, 0:1], mv[:, 0:1], float(ROWS))
                nc.vector.tensor_tensor(
                    out=part[:, 1:2], in0=mv[:, 0:1], in1=mv[:, 0:1], op=ALU.mult
                )
                nc.vector.tensor_tensor(
                    out=part[:, 1:2], in0=part[:, 1:2], in1=mv[:, 1:2], op=ALU.add
                )
                nc.vector.tensor_scalar_mul(part[:, 1:2], part[:, 1:2], float(ROWS))
                nc.sync.dma_start(out=var_in[:], in_=part[:])
                nc.gpsimd.collective_compute(
                    "AllReduce", ALU.add,
                    replica_groups=RG8,
                    ins=[var_in[:].opt()], outs=[var_out[:].opt()],
                )
                nc.sync.dma_start(out=gsum[:], in_=var_out[:])

                # bulk rsqrt for all 64 (tile, subrange) pairs
                nc.scalar.activation(
                    out=std_all[:],
                    in_=mv_all[:].rearrange("p t s d -> p (t s d)")[:, 1::2],
                    func=AF.Sqrt, bias=eps_sb[:],
                )
                nc.vector.reciprocal(out=std_all[:], in_=std_all[:])
                nmur = cpool.tile([128, NTOK * 4], F32, name="nmur")
                nc.vector.tensor_tensor(
                    out=nmur[:],
                    in0=mv_all[:].rearrange("p t s d -> p (t s d)")[:, 0::2],
                    in1=std_all[:], op=ALU.mult,
                )
                nc.vector.tensor_scalar_mul(nmur[:], nmur[:], -1.0)
                for t in range(NTOK):
                    nrm = cpool.tile([128, 576], BF16, tag="nrm", bufs=3)
                    for sr in range(4):
                        lo, hi = (sr * 128, (sr + 1) * 128) if sr < 3 else (384, 576)
                        nc.vector.tensor_scalar(
                            out=nrm[:, lo:hi], in0=qkv_sb[:, t, lo:hi],
                            scalar1=std_all[:, 4 * t + sr:4 * t + sr + 1],
                            scalar2=nmur[:, 4 * t + sr:4 * t + sr + 1],
                            op0=ALU.mult, op1=ALU.add,
                        )
                    nc.vector.tensor_copy(vfull[:, t, 0:192], nrm[:, 384:576])
                    for sr in range(3):
                        ps_tr = cps.tile([128, 128], BF16, tag="tr")
                        nc.tensor.transpose(
                            ps_tr[:], nrm[:, sr * 128:(sr + 1) * 128], ident[:]
                        )
                        av = 0 if sr < 2 else 2
                        nc.scalar.activation(
                            out=qkT[:, sr, t * TOK:(t + 1) * TOK], in_=ps_tr[:],
                            func=AF.Identity, scale=vec_sb[:, av:av + 1],
                            bias=vec_sb[:, av + 1:av + 2],
                        )

                # ---------------- B-scale ----------------
                nc.vector.tensor_scalar_mul(scl[:, 0:1], gsum[:, 0:1], 1.0 / MTOT)
                nc.vector.tensor_tensor(
                    out=scl[:, 0:1], in0=scl[:, 0:1], in1=scl[:, 0:1], op=ALU.mult
                )
                nc.vector.tensor_scalar_mul(scl[:, 1:2], gsum[:, 1:2], 1.0 / MTOT)
                nc.vector.tensor_tensor(
                    out=scl[:, 0:1], in0=scl[:, 1:2], in1=scl[:, 0:1],
                    op=ALU.subtract
                )
                nc.vector.tensor_scalar(
                    out=scl[:, 0:1], in0=scl[:, 0:1], scalar1=MOMENTUM,
                    scalar2=vec_sb[:, 8:9], op0=ALU.mult, op1=ALU.add,
                )
                nc.scalar.activation(out=scl[:, 0:1], in_=scl[:, 0:1], func=AF.Sqrt)
                nc.vector.reciprocal(out=scl[:, 0:1], in_=scl[:, 0:1])
                nc.vector.tensor_tensor(
                    out=scl[:, 0:1], in0=scl[:, 0:1], in1=vec_sb[:, 6:7],
                    op=ALU.mult
                )
                nc.vector.tensor_copy(scl[:, 1:2], vec_sb[:, 7:8])

            # ---------------- B2: gelu + bias projection ----------------
            # A2A in-buffer rows: dup*512 + h*64 + jl_half (jl_half < 64)
            with tc.tile_pool(name="b2", bufs=1) as b2, \
                 tc.tile_pool(name="b2ps", bufs=2, space="PSUM") as b2ps:
                # warm burst gated on scl: re-arms the PE clock right before
                # the projection matmuls start
                wmd = b2.tile([128, 128], BF16, name="wmd")
                nc.vector.tensor_scalar(
                    out=wmd[:], in0=ident[:], scalar1=scl[:, 0:1], scalar2=None,
                    op0=ALU.mult,
                )
                wps2 = b2ps.tile([128, 512], F32, tag="psb", name="wps2")
                for r in range(60):
                    nc.tensor.matmul(
                        wps2[:, 0:128], wmd[:], ident[:], start=True, stop=True
                    )
                for hb in range(8):
                    pt2 = b2.tile([128, 4, 2048], BF16, tag="pw2", bufs=2)
                    nc.sync.dma_start(
                        out=pt2[:], in_=pw_T[:, hb * 8192:(hb + 1) * 8192]
                    )
                    acc = b2.tile([128, 4, 512], BF16, tag="acc", bufs=2)
                    for tl in range(4):
                        gel = b2.tile([128, 2048], BF16, tag="gel", bufs=3)
                        nc.scalar.activation(
                            out=gel[:], in_=pt2[:, tl, :], func=AF.Gelu,
                            bias=scl[:, 1:2], scale=scl[:, 0:1],
                        )
                        ps_b = b2ps.tile([128, 512], F32, tag="psb")
                        # full-tile init: the copy below reads all 128
                        # partitions but only rows 32q..32q+8 are written
                        nc.vector.memset(ps_b[:], 0.0)
                        for q in range(4):
                            nc.tensor.matmul(
                                ps_b[32 * q:32 * q + 8, :], wb_sb[:],
                                gel[:, q * 512:(q + 1) * 512],
                                start=True, stop=True,
                                tile_position=(0, 32 * q),
                            )
                        nc.vector.tensor_copy(acc[:, tl, :], ps_b[:])
                    dst = a2a_in_a if hb < 4 else a2a_in_b
                    for q in range(4):
                        nc.sync.dma_start(
                            out=_ap(
                                dst[(hb % 4) * 16 + q, 0],
                                [[64 * 512, 8], [4 * 512, 4], [1, 512]],
                            ),
                            in_=acc[32 * q:32 * q + 8, :, :],
                        )
                    if hb == 3:
                        nc.sync.dma_start(
                            out=a2a_in_a[512:1024, :], in_=a2a_in_a[0:512, :]
                        )
                        nc.gpsimd.collective_compute(
                            "AllToAll", ALU.bypass,
                            replica_groups=RG8,
                            ins=[a2a_in_a[:].opt()],
                            outs=[a2a_out_a[:].opt()],
                        )
                nc.sync.dma_start(
                    out=a2a_in_b[512:1024, :], in_=a2a_in_b[0:512, :]
                )
                nc.gpsimd.collective_compute(
                    "AllToAll", ALU.bypass,
                    replica_groups=RG8,
                    ins=[a2a_in_b[:].opt()], outs=[a2a_out_b[:].opt()],
                )

            # load E-phase constants (issued late so they don't delay pw/x)
            nc.sync.dma_start(out=wo_sb[:], in_=w_out_c[:])
            nc.sync.dma_start(
                out=bout_bc[:], in_=_ap(b_out_c[:], [[0, 128], [1, OUTC]])
            )

            # ---------------- D: attention, E: out projection ----------------
            with tc.tile_pool(name="dper", bufs=1) as dper, \
                 tc.tile_pool(name="dsb", bufs=2) as dsb, \
                 tc.tile_pool(name="osb", bufs=2) as osb, \
                 tc.tile_pool(name="dps", bufs=2, space="PSUM") as dps, \
                 tc.tile_pool(name="dpo", bufs=1, space="PSUM") as dpo:
                for j in J_ORDER:
                    for g in range(GPC):
                        src_t = a2a_out_a if (j % 4) < 2 else a2a_out_b
                        nc.gpsimd.indirect_dma_start(
                            out=bias_gath[:, g, j, :],
                            out_offset=None,
                            in_=src_t[:],
                            in_offset=bass.IndirectOffsetOnAxis(
                                ap=bidx_sb[:, g * JC + j:g * JC + j + 1], axis=0
                            ),
                        )

                pending = []   # deferred post-processing closures

                def flush_pending():
                    for f in pending:
                        f()
                    pending.clear()

                def d_pass(sp, g, mid_cb=None):
                    i0, ln = SPANS[sp]
                    nh = ln // 512
                    ps = {}
                    stash = []

                    def emit_avs(j, pT, first, last):
                        for h2 in range(nh):
                            nc.tensor.matmul(
                                ps["oa"][:, h2 * 512:(h2 + 1) * 512],
                                vfull[:, j, 0:128],
                                pT[:, h2 * 512:(h2 + 1) * 512],
                                start=first, stop=last,
                            )
                        for h2 in range(nh):
                            nc.tensor.matmul(
                                ps["ob"][:, h2 * 512:(h2 + 1) * 512],
                                vfull[:, j, 128:193],
                                pT[:, h2 * 512:(h2 + 1) * 512],
                                start=first, stop=last,
                            )

                    for jj, j in enumerate(J_ORDER):
                        ps_s = dps.tile([128, 1024], F32, tag="s", name="ps_s")
                        for h2 in range(nh):
                            nc.tensor.matmul(
                                ps_s[:, h2 * 512:(h2 + 1) * 512],
                                qkT[:, 2, j * 128:(j + 1) * 128],
                                qkT[:, g, i0 + h2 * 512:i0 + (h2 + 1) * 512],
                                start=True, stop=True,
                            )
                        bt = bias_gath[:, g, j, i0 // 4:i0 // 4 + ln // 4]
                        bb = _ap(bt, [bt.ap[0], bt.ap[1], [0, 4]])
                        u = dsb.tile([128, 1024], FP16, tag="u", name="u")
                        nc.vector.tensor_tensor(
                            out=u[:, 0:ln].rearrange("p (a b) -> p a b", b=4),
                            in0=ps_s[:, 0:ln].rearrange("p (a b) -> p a b", b=4),
                            in1=bb, op=ALU.add,
                        )
                        ut = dsb.tile([128, 1024], FP16, tag="ut", name="ut")
                        nc.scalar.activation(
                            out=ut[:, 0:ln], in_=u[:, 0:ln], func=AF.Tanh
                        )
                        pT = dsb.tile([128, 1024], BF16, tag="pT", bufs=10,
                                      name="pT")
                        nc.scalar.activation(
                            out=pT[:, 0:ln], in_=ut[:, 0:ln], func=AF.Exp,
                            scale=CLAMP
                        )
                        if jj < 8:
                            stash.append((j, pT))
                        else:
                            if jj == 8:
                                flush_pending()
                                ps["oa"] = dpo.tile([128, 1024], F32, tag="oa",
                                                    name="ps_oa")
                                ps["ob"] = dpo.tile([65, 1024], F32, tag="ob",
                                                    name="ps_ob")
                                if mid_cb is not None:
                                    mid_cb()
                                for k2, (j0, pT0) in enumerate(stash):
                                    emit_avs(j0, pT0, k2 == 0, False)
                            emit_avs(j, pT, False, jj == JC - 1)

                    def post(sp=sp, g=g, ps_oa=ps["oa"], ps_ob=ps["ob"],
                             ln=ln, nh=nh):
                        rd = osb.tile([1, 1024], F32, tag="rd", name="rd")
                        nc.vector.reciprocal(rd[:, 0:ln], ps_ob[64:65, 0:ln])
                        rdb = osb.tile([1, 1024], BF16, tag="rdb", name="rdb")
                        nc.vector.tensor_copy(rdb[:, 0:ln], rd[:, 0:ln])
                        ps_bc = dps.tile([128, 1024], F32, tag="s", name="ps_bc")
                        for h2 in range(nh):
                            nc.tensor.matmul(
                                ps_bc[:, h2 * 512:(h2 + 1) * 512], ones1[:],
                                rdb[:, h2 * 512:(h2 + 1) * 512],
                                start=True, stop=True,
                            )
                        bc_sb = osb.tile([128, 1024], BF16, tag="bc_sb",
                                         name="bc_sb")
                        nc.vector.tensor_copy(bc_sb[:, 0:ln], ps_bc[:, 0:ln])
                        oa = osb.tile([128, 1024], BF16, tag="oa_sb", name="oa")
                        ob = osb.tile([64, 1024], BF16, tag="ob_sb", name="ob")
                        nc.vector.scalar_tensor_tensor(
                            out=oa[:, 0:ln], in0=bc_sb[:, 0:ln],
                            scalar=vwb_sb[:, 0:1], in1=ps_oa[:, 0:ln],
                            op0=ALU.mult, op1=ALU.mult,
                        )
                        nc.vector.tensor_scalar(
                            out=oa[:, 0:ln], in0=oa[:, 0:ln], scalar1=1.0,
                            scalar2=vwb_sb[:, 1:2], op0=ALU.mult, op1=ALU.add,
                        )
                        nc.vector.scalar_tensor_tensor(
                            out=ob[:, 0:ln], in0=bc_sb[0:64, 0:ln],
                            scalar=vwb_sb[0:64, 2:3], in1=ps_ob[0:64, 0:ln],
                            op0=ALU.mult, op1=ALU.mult,
                        )
                        nc.vector.tensor_scalar(
                            out=ob[:, 0:ln], in0=ob[:, 0:ln], scalar1=1.0,
                            scalar2=vwb_sb[0:64, 3:4], op0=ALU.mult, op1=ALU.add,
                        )
                        nc.sync.dma_start(
                            out=ot_own[sp][g * DV:g * DV + 128, :],
                            in_=oa[:, 0:ln]
                        )
                        nc.sync.dma_start(
                            out=ot_own[sp][g * DV + 128:g * DV + DV, :],
                            in_=ob[:, 0:ln]
                        )
                        if g == 1:
                            nc.gpsimd.collective_compute(
                                "AllGather", ALU.bypass,
                                replica_groups=RG4,
                                ins=[ot_own[sp][:].opt()],
                                outs=[ot_all[sp][:].opt()],
                            )

                    pending.append(post)

                def e_pass(sp, tls):
                    i0, ln = SPANS[sp]
                    mge = dper.tile([128, DCH, ln], BF16, name=f"mge{sp}",
                                    tag=f"mge{sp}")
                    nc.sync.dma_start(
                        out=mge[:],
                        in_=_ap(ot_all[sp][0, 0],
                                [[ln, 128], [128 * ln, DCH], [1, ln]]),
                    )
                    for tl in tls:
                        t = i0 // TOK + tl
                        ps_o = dpo.tile(
                            [128, OUTC], F32,
                            tag=("oa" if tl % 2 == 0 else "ob"), name="ps_o",
                        )
                        for kc in range(DCH):
                            nc.tensor.matmul(
                                ps_o[:], mge[:, kc, tl * 128:(tl + 1) * 128],
                                wo_sb[:, kc, :], start=(kc == 0),
                                stop=(kc == DCH - 1),
                            )
                        o_out = osb.tile([128, OUTC], F32, tag="oout",
                                         name="o_out")
                        nc.vector.tensor_tensor(
                            out=o_out[:], in0=ps_o[:], in1=bout_bc[:],
                            op=ALU.add
                        )
                        nc.sync.dma_start(
                            out=out_c[t * TOK:(t + 1) * TOK, :], in_=o_out[:]
                        )

                d_pass(0, 0)
                d_pass(0, 1)
                d_pass(1, 0)
                d_pass(1, 1)
                d_pass(2, 0, mid_cb=lambda: e_pass(0, range(8)))
                d_pass(2, 1, mid_cb=lambda: e_pass(1, range(4)))
                flush_pending()
                e_pass(2, range(4))

    return nc


def prepare_in_maps(inputs):
    bf16 = ml_dtypes.bfloat16
    x = np.asarray(inputs["x"], np.float32)
    pairwise = np.asarray(inputs["pairwise"], np.float32)
    w_qkv = np.asarray(inputs["w_qkv"], np.float32)
    q_w = np.asarray(inputs["q_w"], np.float32)
    q_b = np.asarray(inputs["q_b"], np.float32)
    k_w = np.asarray(inputs["k_w"], np.float32)
    k_b = np.asarray(inputs["k_b"], np.float32)
    v_w = np.asarray(inputs["v_w"], np.float32)
    v_b = np.asarray(inputs["v_b"], np.float32)
    gamma = np.asarray(inputs["bias_gamma"], np.float32)
    beta = np.asarray(inputs["bias_beta"], np.float32)
    rvar = np.asarray(inputs["bias_running_var"], np.float32)
    w_bias = np.asarray(inputs["w_bias"], np.float32)
    w_out = np.asarray(inputs["w_out"], np.float32)
    b_out = np.asarray(inputs["b_out"], np.float32)

    vecs = np.zeros((12, 192), np.float32)
    vecs[0, :128] = q_w * (SCALE / CLAMP)
    vecs[1, :128] = q_b * (SCALE / CLAMP)
    vecs[2, :128] = k_w
    vecs[3, :128] = k_b
    vecs[4, :192] = v_w
    vecs[5, :192] = v_b
    vecs[6, :128] = gamma
    vecs[7, :128] = beta
    vecs[8, :128] = (1.0 - MOMENTUM) * rvar + EPS

    w_bias_e = (w_bias / CLAMP).astype(bf16)

    in_maps = []
    for c in range(NCORES):
        b, a = divmod(c, 4)
        xt = np.ascontiguousarray(
            x[b].T.reshape(DCH, 128, N).transpose(1, 0, 2)
        ).astype(bf16)
        pw = pairwise[b, :, a * JBLK:(a + 1) * JBLK, :]        # [i, jl, dp]
        pw = np.ascontiguousarray(pw.transpose(2, 1, 0).reshape(128, ROWS)
                                  ).astype(bf16)
        qcols = w_qkv[:, 2 * a * DQK:(2 * a + 2) * DQK]
        kcols = w_qkv[:, G * DQK:G * DQK + DQK]
        vcols = w_qkv[:, G * DQK + DQK:]
        wq = np.concatenate([qcols, kcols, vcols], axis=1)     # [1536, 576]
        wq = np.ascontiguousarray(
            wq.reshape(DCH, 128, 576).transpose(1, 0, 2)).astype(bf16)
        wo = w_out[:, a * OUTC:(a + 1) * OUTC]
        wo = np.ascontiguousarray(
            wo.reshape(DCH, 128, OUTC).transpose(1, 0, 2)).astype(bf16)
        # bias gather rows in the A2A out half-buffer [1024, 512]:
        # row = (b*4 + block)*128 + g*64 + jl_half, jl_half = ((j%4)%2)*32 + p//4
        gg, jj, pp = np.meshgrid(
            np.arange(GPC), np.arange(JC), np.arange(128), indexing="ij"
        )
        bidx_np = (
            (b * 4 + jj // 4) * 128 + gg * 64 + ((jj % 4) % 2) * 32 + pp // 4
        ).astype(np.int32)
        in_maps.append({
            "x_T": xt,
            "pw_T": pw,
            "w_qkv_c": wq,
            "w_bias_e": w_bias_e,
            "w_out_c": wo,
            "b_out_c": b_out[None, a * OUTC:(a + 1) * OUTC].astype(np.float32),
            "vecs": vecs,
            "bidx": bidx_np,
        })
    return in_maps


_NC_CACHE = None


def _get_nc():
    global _NC_CACHE
    if _NC_CACHE is None:
        _NC_CACHE = build_graph()
    return _NC_CACHE


def kernel(**inputs):
    from concourse.bass_utils import run_bass_kernel_spmd

    in_maps = prepare_in_maps(inputs)
    nc = _get_nc()
    res = run_bass_kernel_spmd(
        nc, in_maps, core_ids=list(range(NCORES)),
        trace=bool(int(os.environ.get("BASS_KERNEL_TRACE", "0"))),
        tmpdir=os.environ.get("BASS_KERNEL_TMPDIR"),
    )
    if res.exec_time_ns is not None:
        print(f"HW exec time: {res.exec_time_ns} ns", file=sys.stderr)

    out = np.zeros((B, N, D), np.float32)
    for c in range(NCORES):
        b, a = divmod(c, 4)
        out[b, :, a * OUTC:(a + 1) * OUTC] = res.results[c]["out_c"]
    return out


# revision 20
# speedup vs baseline: 1.2394x; 1.0664x over previous
"""Distributed Trainium2 Bass kernel for nn_Attention_57243324121446.

GQA attention (8 query groups, 1 kv head) with a pairwise-bias branch
(BatchRMSNorm -> exact gelu -> head projection, 4x nearest-neighbor upsample),
softclamp tanh, softmax, out-projection.

Sharding (8 cores): core c -> batch b = c//4, query groups {2*(c%4), 2*(c%4)+1}.
k/v are computed redundantly per core (single shared kv head). Pairwise is
sharded by (b, coarse-j block of 128 rows).

v2 layout (optimized):
 - Phase C (qkv+LN) overlaps phase B1 (pairwise stats streaming); the first
   16 of 32 pairwise tiles are cached in SBUF so B2 re-reads only half.
 - Bias exchange is an 8-rank AllToAll with duplicated head-pair chunks
   (wire ~0.9MB/half vs 3.7MB for the old 8-rank AllGather).
 - Attention is computed S^T = K^T q per j-chunk; P^T tiles feed AV matmuls
   as the *stationary* operand so the output lands as o[i, dv] with the
   softmax denominator accumulated for free in column 192 (ones column of v).
 - o is normalized per-i-partition (cheap [128,8] reciprocal), transposed via
   the PE into o^T, and AllGather'ed within the 4-core batch group per
   1024-token i-span; the out-projection for i-span 0 runs while span 1 is
   still computing.
"""

import os
import sys

sys.path.insert(0, "/opt/trn_rl_repo")

import numpy as np
import ml_dtypes

import concourse.bass as bass
import concourse.mybir as mybir
import concourse.tile as tile
from concourse.masks import make_identity


# --- workaround: this container's walrus caps CTRL instructions at 2 sem
# waits; Tile's kernel-tail drain can carry many. Split them across drains.
def _patched_drain_and_barrier(self, tick_clock, wait_clock):
    from concourse.vector_clock import ScopedClock
    drain_inst = self.nc.sync.drain()
    wait_clock.add_sem_waits(
        drain_inst.ins, ScopedClock({None: tick_clock.global_clock})
    )
    si = drain_inst.ins.sync_info
    if si is not None and len(si.on_wait) > 1:
        waits = list(si.on_wait)
        drain_inst.ins.sync_info = mybir.SyncInfo(
            on_wait=waits[:1], on_update=list(si.on_update)
        )
        for i in range(1, len(waits)):
            extra = self.nc.sync.drain()
            extra.ins.sync_info = mybir.SyncInfo(
                on_wait=waits[i:i + 1], on_update=[]
            )
    self.nc.all_engine_barrier()
    assert self.sems is not None
    popped = self.nc._tile_sem_poison_stack.pop()
    assert popped is self._sem_poison
    self.nc.clear_and_free_semaphores(list(self.sems.allocated().values()))
    self.nc.all_engine_barrier()


tile.TileContext._drain_and_barrier = _patched_drain_and_barrier


# --- workaround 2: this walrus accepts at most ONE sem wait per instruction.
# Rewrite the BIR json before compile: hoist excess waits onto same-engine
# Nop carriers inserted immediately before the offending instruction.
import json as _json
import concourse.bass_utils as _bass_utils
import concourse.bass2jax as _bass2jax


def _split_bir_multiwaits(bir_json):
    d = _json.loads(bir_json)
    mods = d.get("modules") or [d]
    for m in mods:
        for fn in m.get("functions", []):
            for bb in fn.get("blocks", []):
                out = []
                changed = False
                for ins in bb["instructions"]:
                    si = ins.get("sync_info")
                    w = (si or {}).get("on_wait") or []
                    if len(w) > 1 and ins.get("engine"):
                        eng = ins["engine"]
                        for i, wi in enumerate(w[:-1]):
                            out.append({
                                "debug": ins.get("debug"),
                                "engine": eng,
                                "ins": [{"dtype": "int32", "kind": "imm_value",
                                         "value": 0}],
                                "name": ins["name"] + f".sw{i}",
                                "opcode": "RegisterMove",
                                "outs": [{"dtype": "int32",
                                          "kind": "register_access",
                                          "regref": f"{eng}_zero"}],
                                "sync_info": {"on_update": [], "on_wait": [wi]},
                            })
                        si["on_wait"] = [w[-1]]
                        changed = True
                    out.append(ins)
                if changed:
                    bb["instructions"] = out
    return _json.dumps(d).encode()


_orig_compile_bir = _bass_utils.compile_bir_kernel


def _patched_compile_bir(bir_json, tmpdir, neff_name="file.neff"):
    return _orig_compile_bir(_split_bir_multiwaits(bir_json), tmpdir, neff_name)


_bass_utils.compile_bir_kernel = _patched_compile_bir
_bass2jax.compile_bir_kernel = _patched_compile_bir


# --- workaround 3: the agent image's antenv lacks axon_hooks, so the boot
# shim never registers the NTFF profile hook. Provide the module and install
# the ctypes hook ourselves so run_bass_kernel_spmd(trace=True) works.
def _install_ntff_hook():
    import types as _types
    mod = sys.modules.get("antenv.axon_hooks")
    if mod is None:
        mod = _types.ModuleType("antenv.axon_hooks")
        mod._hook = None
        def _set(h):
            mod._hook = h
        def _get():
            return mod._hook
        mod.set_axon_ntff_profile_hook = _set
        mod.get_axon_ntff_profile_hook = _get
        sys.modules["antenv.axon_hooks"] = mod
        import antenv as _antenv
        _antenv.axon_hooks = mod
    if mod._hook is None and os.path.exists("/opt/axon/libaxon_pjrt.so"):
        try:
            from trn_agent_boot.trn_boot import _ntff_profile_via_ctypes
            mod._hook = _ntff_profile_via_ctypes("/opt/axon/libaxon_pjrt.so")
        except Exception as e:
            print(f"ntff hook install failed: {e}", file=sys.stderr)


_install_ntff_hook()


BF16 = mybir.dt.bfloat16
FP16 = mybir.dt.float16
F32 = mybir.dt.float32
AF = mybir.ActivationFunctionType
ALU = mybir.AluOpType

B, N, D = 2, 2048, 1536
HEADS, KVH, DQK, DV = 8, 1, 128, 192
G = HEADS // KVH
NP, DP = 512, 128
SCALE = DQK ** -0.5
CLAMP = 5.0
MOMENTUM = 0.1
EPS = 1e-5

NCORES = 8
GPC = 2              # query groups per core
JBLK = NP // 4       # pairwise coarse-j rows per core = 128
ROWS = JBLK * NP     # pairwise rows per core = 65536
TOK = 128            # token chunk
NTOK = N // TOK      # 16
DCH = D // 128       # 12 d-model chunks
JC = N // 128        # 16 fine-j chunks
OUTC = D // 4        # 384 out cols per core
NPW = 32             # pairwise tiles of 2048 rows
KCACHE = 16          # pairwise tiles kept in SBUF between B1 and B2
MTOT = float(B * NP * NP)
HSP = N // 2         # i-span = 1024
J_ORDER = [j for j in range(JC) if j % 4 < 2] + [j for j in range(JC) if j % 4 >= 2]
RG8 = [list(range(NCORES))]
RG4 = [[0, 1, 2, 3], [4, 5, 6, 7]]


def _ap(base, dims):
    return bass.AP(tensor=base.tensor, offset=base.offset, ap=dims)


def build_graph():
    nc = bass.Bass()

    x_T = nc.declare_dram_parameter("x_T", [128, DCH, N], BF16, isOutput=False)
    pw_T = nc.declare_dram_parameter("pw_T", [128, ROWS], BF16, isOutput=False)
    w_qkv_c = nc.declare_dram_parameter("w_qkv_c", [128, DCH, 576], BF16, isOutput=False)
    w_bias_e = nc.declare_dram_parameter("w_bias_e", [128, 8], BF16, isOutput=False)
    w_out_c = nc.declare_dram_parameter("w_out_c", [128, DCH, OUTC], BF16, isOutput=False)
    b_out_c = nc.declare_dram_parameter("b_out_c", [1, OUTC], F32, isOutput=False)
    # vecs rows: 0 qw_eff,1 qb_eff,2 kw,3 kb,4 vw(192),5 vb(192),6 gamma,
    #            7 beta,8 rv9eps
    vecs = nc.declare_dram_parameter("vecs", [12, 192], F32, isOutput=False)
    bidx = nc.declare_dram_parameter("bidx", [GPC, JC, 128], mybir.dt.int32, isOutput=False)
    out_c = nc.declare_dram_parameter("out_c", [N, OUTC], F32, isOutput=True)

    with tile.TileContext(nc) as tc:
        with tc.tile_pool(name="const", bufs=1) as const, \
             tc.tile_pool(name="dram", bufs=1, space="DRAM") as dram:

            # ---------------- constants ----------------
            ident = const.tile([128, 128], BF16)
            make_identity(nc, ident[:])
            vec_sb = const.tile([128, 12], F32)
            nc.sync.dma_start(out=vec_sb[:], in_=_ap(vecs[:], [[1, 128], [192, 12]]))
            # vwb plane 0 = v_w broadcast, plane 1 = v_b broadcast (free dim)
            ones1 = const.tile([1, 128], BF16)
            nc.vector.memset(ones1[:], 1.0)
            # vwb_sb: col0 vw[0:128], col1 vb[0:128], col2 vw[128:192], col3 vb
            vwb_sb = const.tile([128, 4], F32)
            nc.sync.dma_start(
                out=vwb_sb[:, 0:2], in_=_ap(vecs[4, 0], [[1, 128], [192, 2]])
            )
            nc.sync.dma_start(
                out=vwb_sb[0:64, 2:4], in_=_ap(vecs[4, 128], [[1, 64], [192, 2]])
            )
            eps_sb = const.tile([128, 1], F32)
            nc.vector.memset(eps_sb[:], EPS)
            wq_sb = const.tile([128, DCH, 576], BF16)
            nc.sync.dma_start(out=wq_sb[:], in_=w_qkv_c[:])
            wb_sb = const.tile([128, 8], BF16)
            nc.sync.dma_start(out=wb_sb[:], in_=w_bias_e[:])
            bidx_sb = const.tile([128, GPC * JC], mybir.dt.int32)
            nc.sync.dma_start(
                out=bidx_sb[:], in_=_ap(bidx[:], [[1, 128], [128, GPC * JC]])
            )

            bias_gath = const.tile([128, GPC, JC, 512], BF16)
            qkT = const.tile([128, 3, N], BF16)       # q0^T, q1^T, k^T
            vfull = const.tile([128, NTOK, 208], BF16)  # v~*vw cols 0:192, 192=1
            nc.vector.memset(vfull[:], 0.0)
            nc.vector.memset(vfull[:, :, 192:193], 1.0)
            stats = const.tile([128, 8, 16, 6], F32)
            mv = const.tile([128, 2], F32)
            part = const.tile([128, 2], F32)
            gsum = const.tile([128, 2], F32)
            scl = const.tile([128, 2], F32)
            wo_sb = const.tile([128, DCH, OUTC], BF16)
            bout_bc = const.tile([128, OUTC], F32)

            var_in = dram.tile([128, 2], F32)
            var_out = dram.tile([128, 2], F32, addr_space="Shared")
            a2a_in_a = dram.tile([1024, 512], BF16)
            a2a_in_b = dram.tile([1024, 512], BF16)
            a2a_out_a = dram.tile([1024, 512], BF16)
            a2a_out_b = dram.tile([1024, 512], BF16)
            SPANS = [(0, 1024), (1024, 512), (1536, 512)]
            ot_own = [[dram.tile([DV, ln], BF16, name=f"oto{i}{g}")
                       for g in range(2)] for i, (o, ln) in enumerate(SPANS)]
            ot_all = [[dram.tile([4 * DV, ln], BF16, name=f"ota{i}{g}")
                       for g in range(2)] for i, (o, ln) in enumerate(SPANS)]

            # ---------------- PE warmup (HAM unthrottle) ----------------
            with tc.tile_pool(name="warm", bufs=1) as warm, \
                 tc.tile_pool(name="warmps", bufs=1, space="PSUM") as warmps:
                wps = warmps.tile([128, 512], F32)
                wsb = warm.tile([128, 128], F32)
                wdr = dram.tile([128, 128], F32)
                for r in range(100):
                    nc.tensor.matmul(
                        wps[:, 0:128], ident[:], ident[:],
                        start=True, stop=True,
                    )
                nc.vector.tensor_copy(wsb[:, 0:128], wps[:, 0:128])
                nc.sync.dma_start(out=wdr[:], in_=wsb[:])

            # ---------------- B1 (pairwise stats) + C (qkv/LN), overlapped ----------------
            with tc.tile_pool(name="cpool", bufs=1) as cpool, \
                 tc.tile_pool(name="cps", bufs=2, space="PSUM") as cps:
                xt_sb = cpool.tile([128, DCH, N], BF16, name="xt_sb")
                nc.sync.dma_start(out=xt_sb[:], in_=x_T[:])
                qkv_sb = cpool.tile([128, NTOK, 576], BF16, name="qkv_sb")
                st_all = cpool.tile([128, NTOK, 4, 6], F32, name="st_all")
                mv_all = cpool.tile([128, NTOK, 4, 2], F32, name="mv_all")
                std_all = cpool.tile([128, NTOK * 4], F32, name="std_all")
                pwt = [None] * 8
                for tb in range(8):
                    pwt[tb] = cpool.tile([128, 4, 2048], BF16, tag="pw", bufs=3,
                                         name="pwt")
                    nc.sync.dma_start(
                        out=pwt[tb][:], in_=pw_T[:, tb * 8192:(tb + 1) * 8192]
                    )
                for t in range(NTOK):
                    ps_qkv = cps.tile([128, 576], F32, tag="qkv")
                    for c in range(DCH):
                        nc.tensor.matmul(
                            ps_qkv[:, 0:512], xt_sb[:, c, t * TOK:(t + 1) * TOK],
                            wq_sb[:, c, 0:512], start=(c == 0),
                            stop=(c == DCH - 1),
                        )
                        nc.tensor.matmul(
                            ps_qkv[:, 512:576], xt_sb[:, c, t * TOK:(t + 1) * TOK],
                            wq_sb[:, c, 512:576], start=(c == 0),
                            stop=(c == DCH - 1),
                        )
                    nc.scalar.copy(qkv_sb[:, t, :], ps_qkv[:])
                    # interleave B1 stats: 8 per C tile (pw big-tile t//2)
                    tb, hq = t // 2, (t % 2) * 8
                    for q2 in range(hq, hq + 8):
                        nc.vector.bn_stats(
                            out=stats[:, tb, q2, :],
                            in_=pwt[tb][:, q2 // 4, (q2 % 4) * 512:(q2 % 4 + 1) * 512],
                        )
                # aggregate + AllReduce as soon as stats are done
                nc.vector.bn_aggr(
                    out=mv[:], in_=stats[:].rearrange("p a b c -> p (a b) c")
                )
                nc.vector.tensor_scalar_mul(part[:, 0:1], mv[:, 0:1], float(ROWS))
                nc.vector.tensor_tensor(
                    out=part[:, 1:2], in0=mv[:, 0:1], in1=mv[:, 0:1], op=ALU.mult
                )
                nc.vector.tensor_tensor(
                    out=part[:, 1:2], in0=part[:, 1:2], in1=mv[:, 1:2], op=ALU.add
                )
                nc.vector.tensor_scalar_mul(part[:, 1:2], part[:, 1:2], float(ROWS))
                nc.sync.dma_start(out=var_in[:], in_=part[:])
                nc.gpsimd.collective_compute(
                    "AllReduce", ALU.add,
                    replica_groups=RG8,
                    ins=[var_in[:].opt()], outs=[var_out[:].opt()],
                )
                nc.sync.dma_start(out=gsum[:], in_=var_out[:])

                # C LN stats (after the AllReduce trigger in the DVE queue)
                for t in range(NTOK):
                    for sr in range(4):
                        lo, hi = (sr * 128, (sr + 1) * 128) if sr < 3 else (384, 576)
                        nc.vector.bn_stats(
                            out=st_all[:, t, sr, :], in_=qkv_sb[:, t, lo:hi]
                        )
                        nc.vector.bn_aggr(
                            out=mv_all[:, t, sr, :], in_=st_all[:, t, sr, :]
                        )
                # bulk rsqrt for all 64 (tile, subrange) pairs
                nc.scalar.activation(
                    out=std_all[:],
                    in_=mv_all[:].rearrange("p t s d -> p (t s d)")[:, 1::2],
                    func=AF.Sqrt, bias=eps_sb[:],
                )
                nc.vector.reciprocal(out=std_all[:], in_=std_all[:])
                nmur = cpool.tile([128, NTOK * 4], F32, name="nmur")
                nc.vector.tensor_tensor(
                    out=nmur[:],
                    in0=mv_all[:].rearrange("p t s d -> p (t s d)")[:, 0::2],
                    in1=std_all[:], op=ALU.mult,
                )
                nc.vector.tensor_scalar_mul(nmur[:], nmur[:], -1.0)
                for t in range(NTOK):
                    nrm = cpool.tile([128, 576], BF16, tag="nrm", bufs=3)
                    for sr in range(4):
                        lo, hi = (sr * 128, (sr + 1) * 128) if sr < 3 else (384, 576)
                        nc.vector.tensor_scalar(
                            out=nrm[:, lo:hi], in0=qkv_sb[:, t, lo:hi],
                            scalar1=std_all[:, 4 * t + sr:4 * t + sr + 1],
                            scalar2=nmur[:, 4 * t + sr:4 * t + sr + 1],
                            op0=ALU.mult, op1=ALU.add,
                        )
                    nc.vector.tensor_copy(vfull[:, t, 0:192], nrm[:, 384:576])
                    for sr in range(3):
                        ps_tr = cps.tile([128, 128], BF16, tag="tr")
                        nc.tensor.transpose(
                            ps_tr[:], nrm[:, sr * 128:(sr + 1) * 128], ident[:]
                        )
                        av = 0 if sr < 2 else 2
                        nc.scalar.activation(
                            out=qkT[:, sr, t * TOK:(t + 1) * TOK], in_=ps_tr[:],
                            func=AF.Identity, scale=vec_sb[:, av:av + 1],
                            bias=vec_sb[:, av + 1:av + 2],
                        )

                # ---------------- B-scale ----------------
                nc.vector.tensor_scalar_mul(scl[:, 0:1], gsum[:, 0:1], 1.0 / MTOT)
                nc.vector.tensor_tensor(
                    out=scl[:, 0:1], in0=scl[:, 0:1], in1=scl[:, 0:1], op=ALU.mult
                )
                nc.vector.tensor_scalar_mul(scl[:, 1:2], gsum[:, 1:2], 1.0 / MTOT)
                nc.vector.tensor_tensor(
                    out=scl[:, 0:1], in0=scl[:, 1:2], in1=scl[:, 0:1],
                    op=ALU.subtract
                )
                nc.vector.tensor_scalar(
                    out=scl[:, 0:1], in0=scl[:, 0:1], scalar1=MOMENTUM,
                    scalar2=vec_sb[:, 8:9], op0=ALU.mult, op1=ALU.add,
                )
                nc.scalar.activation(out=scl[:, 0:1], in_=scl[:, 0:1], func=AF.Sqrt)
                nc.vector.reciprocal(out=scl[:, 0:1], in_=scl[:, 0:1])
                nc.vector.tensor_tensor(
                    out=scl[:, 0:1], in0=scl[:, 0:1], in1=vec_sb[:, 6:7],
                    op=ALU.mult
                )
                nc.vector.tensor_copy(scl[:, 1:2], vec_sb[:, 7:8])

            # ---------------- B2: gelu + bias projection ----------------
            # A2A in-buffer rows: dup*512 + h*64 + jl_half (jl_half < 64)
            with tc.tile_pool(name="b2", bufs=1) as b2, \
                 tc.tile_pool(name="b2ps", bufs=2, space="PSUM") as b2ps:
                # warm burst gated on scl: re-arms the PE clock right before
                # the projection matmuls start
                wmd = b2.tile([128, 128], BF16, name="wmd")
                nc.vector.tensor_scalar(
                    out=wmd[:], in0=ident[:], scalar1=scl[:, 0:1], scalar2=None,
                    op0=ALU.mult,
                )
                wps2 = b2ps.tile([128, 512], F32, tag="psb", name="wps2")
                for r in range(60):
                    nc.tensor.matmul(
                        wps2[:, 0:128], wmd[:], ident[:], start=True, stop=True
                    )
                for hb in range(8):
                    pt2 = b2.tile([128, 4, 2048], BF16, tag="pw2", bufs=2)
                    nc.sync.dma_start(
                        out=pt2[:], in_=pw_T[:, hb * 8192:(hb + 1) * 8192]
                    )
                    acc = b2.tile([128, 4, 512], BF16, tag="acc", bufs=2)
                    for tl in range(4):
                        gel = b2.tile([128, 2048], BF16, tag="gel", bufs=3)
                        nc.scalar.activation(
                            out=gel[:], in_=pt2[:, tl, :], func=AF.Gelu,
                            bias=scl[:, 1:2], scale=scl[:, 0:1],
                        )
                        ps_b = b2ps.tile([128, 512], F32, tag="psb")
                        # full-tile init: the copy below reads all 128
                        # partitions but only rows 32q..32q+8 are written
                        nc.vector.memset(ps_b[:], 0.0)
                        for q in range(4):
                            nc.tensor.matmul(
                                ps_b[32 * q:32 * q + 8, :], wb_sb[:],
                                gel[:, q * 512:(q + 1) * 512],
                                start=True, stop=True,
                                tile_position=(0, 32 * q),
                            )
                        nc.vector.tensor_copy(acc[:, tl, :], ps_b[:])
                    dst = a2a_in_a if hb < 4 else a2a_in_b
                    for q in range(4):
                        nc.sync.dma_start(
                            out=_ap(
                                dst[(hb % 4) * 16 + q, 0],
                                [[64 * 512, 8], [4 * 512, 4], [1, 512]],
                            ),
                            in_=acc[32 * q:32 * q + 8, :, :],
                        )
                    if hb == 3:
                        nc.sync.dma_start(
                            out=a2a_in_a[512:1024, :], in_=a2a_in_a[0:512, :]
                        )
                        nc.gpsimd.collective_compute(
                            "AllToAll", ALU.bypass,
                            replica_groups=RG8,
                            ins=[a2a_in_a[:].opt()],
                            outs=[a2a_out_a[:].opt()],
                        )
                nc.sync.dma_start(
                    out=a2a_in_b[512:1024, :], in_=a2a_in_b[0:512, :]
                )
                nc.gpsimd.collective_compute(
                    "AllToAll", ALU.bypass,
                    replica_groups=RG8,
                    ins=[a2a_in_b[:].opt()], outs=[a2a_out_b[:].opt()],
                )

            # load E-phase constants (issued late so they don't delay pw/x)
            nc.sync.dma_start(out=wo_sb[:], in_=w_out_c[:])
            nc.sync.dma_start(
                out=bout_bc[:], in_=_ap(b_out_c[:], [[0, 128], [1, OUTC]])
            )

            # ---------------- D: attention, E: out projection ----------------
            with tc.tile_pool(name="dper", bufs=1) as dper, \
                 tc.tile_pool(name="dsb", bufs=2) as dsb, \
                 tc.tile_pool(name="osb", bufs=2) as osb, \
                 tc.tile_pool(name="dps", bufs=2, space="PSUM") as dps, \
                 tc.tile_pool(name="dpo", bufs=1, space="PSUM") as dpo:
                for j in J_ORDER:
                    for g in range(GPC):
                        src_t = a2a_out_a if (j % 4) < 2 else a2a_out_b
                        nc.gpsimd.indirect_dma_start(
                            out=bias_gath[:, g, j, :],
                            out_offset=None,
                            in_=src_t[:],
                            in_offset=bass.IndirectOffsetOnAxis(
                                ap=bidx_sb[:, g * JC + j:g * JC + j + 1], axis=0
                            ),
                        )

                pending = []   # deferred post-processing closures

                def flush_pending():
                    for f in pending:
                        f()
                    pending.clear()

                def d_pass(sp, g, mid_cb=None):
                    i0, ln = SPANS[sp]
                    nh = ln // 512
                    ps = {}
                    stash = []

                    def emit_avs(j, pT, first, last):
                        for h2 in range(nh):
                            nc.tensor.matmul(
                                ps["oa"][:, h2 * 512:(h2 + 1) * 512],
                                vfull[:, j, 0:128],
                                pT[:, h2 * 512:(h2 + 1) * 512],
                                start=first, stop=last,
                            )
                        for h2 in range(nh):
                            nc.tensor.matmul(
                                ps["ob"][:, h2 * 512:(h2 + 1) * 512],
                                vfull[:, j, 128:193],
                                pT[:, h2 * 512:(h2 + 1) * 512],
                                start=first, stop=last,
                            )

                    for jj, j in enumerate(J_ORDER):
                        ps_s = dps.tile([128, 1024], F32, tag="s", name="ps_s")
                        for h2 in range(nh):
                            nc.tensor.matmul(
                                ps_s[:, h2 * 512:(h2 + 1) * 512],
                                qkT[:, 2, j * 128:(j + 1) * 128],
                                qkT[:, g, i0 + h2 * 512:i0 + (h2 + 1) * 512],
                                start=True, stop=True,
                            )
                        bt = bias_gath[:, g, j, i0 // 4:i0 // 4 + ln // 4]
                        bb = _ap(bt, [bt.ap[0], bt.ap[1], [0, 4]])
                        u = dsb.tile([128, 1024], FP16, tag="u", name="u")
                        nc.vector.tensor_tensor(
                            out=u[:, 0:ln].rearrange("p (a b) -> p a b", b=4),
                            in0=ps_s[:, 0:ln].rearrange("p (a b) -> p a b", b=4),
                            in1=bb, op=ALU.add,
                        )
                        ut = dsb.tile([128, 1024], FP16, tag="ut", name="ut")
                        nc.scalar.activation(
                            out=ut[:, 0:ln], in_=u[:, 0:ln], func=AF.Tanh
                        )
                        pT = dsb.tile([128, 1024], BF16, tag="pT", bufs=10,
                                      name="pT")
                        nc.scalar.activation(
                            out=pT[:, 0:ln], in_=ut[:, 0:ln], func=AF.Exp,
                            scale=CLAMP
                        )
                        if jj < 8:
                            stash.append((j, pT))
                        else:
                            if jj == 8:
                                flush_pending()
                                ps["oa"] = dpo.tile([128, 1024], F32, tag="oa",
                                                    name="ps_oa")
                                ps["ob"] = dpo.tile([65, 1024], F32, tag="ob",
                                                    name="ps_ob")
                                if mid_cb is not None:
                                    mid_cb()
                                for k2, (j0, pT0) in enumerate(stash):
                                    emit_avs(j0, pT0, k2 == 0, False)
                            emit_avs(j, pT, False, jj == JC - 1)

                    def post(sp=sp, g=g, ps_oa=ps["oa"], ps_ob=ps["ob"],
                             ln=ln, nh=nh):
                        rd = osb.tile([1, 1024], F32, tag="rd", name="rd")
                        nc.vector.reciprocal(rd[:, 0:ln], ps_ob[64:65, 0:ln])
                        rdb = osb.tile([1, 1024], BF16, tag="rdb", name="rdb")
                        nc.vector.tensor_copy(rdb[:, 0:ln], rd[:, 0:ln])
                        ps_bc = dps.tile([128, 1024], F32, tag="s", name="ps_bc")
                        for h2 in range(nh):
                            nc.tensor.matmul(
                                ps_bc[:, h2 * 512:(h2 + 1) * 512], ones1[:],
                                rdb[:, h2 * 512:(h2 + 1) * 512],
                                start=True, stop=True,
                            )
                        bc_sb = osb.tile([128, 1024], BF16, tag="bc_sb",
                                         name="bc_sb")
                        nc.vector.tensor_copy(bc_sb[:, 0:ln], ps_bc[:, 0:ln])
                        oa = osb.tile([128, 1024], BF16, tag="oa_sb", name="oa")
                        ob = osb.tile([64, 1024], BF16, tag="ob_sb", name="ob")
                        nc.vector.scalar_tensor_tensor(
                            out=oa[:, 0:ln], in0=bc_sb[:, 0:ln],
                            scalar=vwb_sb[:, 0:1], in1=ps_oa[:, 0:ln],
                            op0=ALU.mult, op1=ALU.mult,
                        )
                        nc.vector.tensor_scalar(
                            out=oa[:, 0:ln], in0=oa[:, 0:ln], scalar1=1.0,
                            scalar2=vwb_sb[:, 1:2], op0=ALU.mult, op1=ALU.add,
                        )
                        nc.vector.scalar_tensor_tensor(
                            out=ob[:, 0:ln], in0=bc_sb[0:64, 0:ln],
                            scalar=vwb_sb[0:64, 2:3], in1=ps_ob[0:64, 0:ln],
                            op0=ALU.mult, op1=ALU.mult,
                        )
                        nc.vector.tensor_scalar(
                            out=ob[:, 0:ln], in0=ob[:, 0:ln], scalar1=1.0,
                            scalar2=vwb_sb[0:64, 3:4], op0=ALU.mult, op1=ALU.add,
                        )
                        nc.sync.dma_start(
                            out=ot_own[sp][g][0:128, :], in_=oa[:, 0:ln]
                        )
                        nc.sync.dma_start(
                            out=ot_own[sp][g][128:DV, :], in_=ob[:, 0:ln]
                        )
                        nc.gpsimd.collective_compute(
                            "AllGather", ALU.bypass,
                            replica_groups=RG4,
                            ins=[ot_own[sp][g][:].opt()],
                            outs=[ot_all[sp][g][:].opt()],
                        )

                    pending.append(post)

                def e_pass(sp, tls):
                    i0, ln = SPANS[sp]
                    mge = dper.tile([128, DCH, ln], BF16, name=f"mge{sp}",
                                    tag=f"mge{sp}")
                    # merged^T row r = kc*128+p = h*192+dv; head h lives in
                    # ot_all[sp][h%2] rows (h//2)*192+dv
                    for h in range(8):
                        src = ot_all[sp][h % 2]
                        r0 = h * DV
                        a0 = (h // 2) * DV
                        cuts = [r0, ((r0 + 127) // 128) * 128, r0 + DV]
                        if cuts[1] == cuts[0]:
                            cuts = [r0, r0 + 128, r0 + DV]
                        for ci in range(len(cuts) - 1):
                            lo, hi = cuts[ci], cuts[ci + 1]
                            nc.sync.dma_start(
                                out=mge[lo % 128:(lo % 128) + (hi - lo),
                                        lo // 128, :],
                                in_=src[a0 + lo - r0:a0 + hi - r0, :],
                            )
                    for tl in tls:
                        t = i0 // TOK + tl
                        ps_o = dpo.tile(
                            [128, OUTC], F32,
                            tag=("oa" if tl % 2 == 0 else "ob"), name="ps_o",
                        )
                        for kc in range(DCH):
                            nc.tensor.matmul(
                                ps_o[:], mge[:, kc, tl * 128:(tl + 1) * 128],
                                wo_sb[:, kc, :], start=(kc == 0),
                                stop=(kc == DCH - 1),
                            )
                        o_out = osb.tile([128, OUTC], F32, tag="oout",
                                         name="o_out")
                        nc.vector.tensor_tensor(
                            out=o_out[:], in0=ps_o[:], in1=bout_bc[:],
                            op=ALU.add
                        )
                        nc.sync.dma_start(
                            out=out_c[t * TOK:(t + 1) * TOK, :], in_=o_out[:]
                        )

                d_pass(0, 0)
                d_pass(0, 1)
                d_pass(1, 0)
                d_pass(1, 1)
                d_pass(2, 0, mid_cb=lambda: e_pass(0, range(8)))
                d_pass(2, 1, mid_cb=lambda: e_pass(1, range(4)))
                flush_pending()
                e_pass(2, range(4))


    return nc


def prepare_in_maps(inputs):
    bf16 = ml_dtypes.bfloat16
    x = np.asarray(inputs["x"], np.float32)
    pairwise = np.asarray(inputs["pairwise"], np.float32)
    w_qkv = np.asarray(inputs["w_qkv"], np.float32)
    q_w = np.asarray(inputs["q_w"], np.float32)
    q_b = np.asarray(inputs["q_b"], np.float32)
    k_w = np.asarray(inputs["k_w"], np.float32)
    k_b = np.asarray(inputs["k_b"], np.float32)
    v_w = np.asarray(inputs["v_w"], np.float32)
    v_b = np.asarray(inputs["v_b"], np.float32)
    gamma = np.asarray(inputs["bias_gamma"], np.float32)
    beta = np.asarray(inputs["bias_beta"], np.float32)
    rvar = np.asarray(inputs["bias_running_var"], np.float32)
    w_bias = np.asarray(inputs["w_bias"], np.float32)
    w_out = np.asarray(inputs["w_out"], np.float32)
    b_out = np.asarray(inputs["b_out"], np.float32)

    vecs = np.zeros((12, 192), np.float32)
    vecs[0, :128] = q_w * (SCALE / CLAMP)
    vecs[1, :128] = q_b * (SCALE / CLAMP)
    vecs[2, :128] = k_w
    vecs[3, :128] = k_b
    vecs[4, :192] = v_w
    vecs[5, :192] = v_b
    vecs[6, :128] = gamma
    vecs[7, :128] = beta
    vecs[8, :128] = (1.0 - MOMENTUM) * rvar + EPS

    w_bias_e = (w_bias / CLAMP).astype(bf16)

    in_maps = []
    for c in range(NCORES):
        b, a = divmod(c, 4)
        xt = np.ascontiguousarray(
            x[b].T.reshape(DCH, 128, N).transpose(1, 0, 2)
        ).astype(bf16)
        pw = pairwise[b, :, a * JBLK:(a + 1) * JBLK, :]        # [i, jl, dp]
        pw = np.ascontiguousarray(pw.transpose(2, 1, 0).reshape(128, ROWS)
                                  ).astype(bf16)
        qcols = w_qkv[:, 2 * a * DQK:(2 * a + 2) * DQK]
        kcols = w_qkv[:, G * DQK:G * DQK + DQK]
        vcols = w_qkv[:, G * DQK + DQK:]
        wq = np.concatenate([qcols, kcols, vcols], axis=1)     # [1536, 576]
        wq = np.ascontiguousarray(
            wq.reshape(DCH, 128, 576).transpose(1, 0, 2)).astype(bf16)
        wo = w_out[:, a * OUTC:(a + 1) * OUTC]
        wo = np.ascontiguousarray(
            wo.reshape(DCH, 128, OUTC).transpose(1, 0, 2)).astype(bf16)
        # bias gather rows in the A2A out half-buffer [1024, 512]:
        # row = (b*4 + block)*128 + g*64 + jl_half, jl_half = ((j%4)%2)*32 + p//4
        gg, jj, pp = np.meshgrid(
            np.arange(GPC), np.arange(JC), np.arange(128), indexing="ij"
        )
        bidx_np = (
            (b * 4 + jj // 4) * 128 + gg * 64 + ((jj % 4) % 2) * 32 + pp // 4
        ).astype(np.int32)
        in_maps.append({
            "x_T": xt,
            "pw_T": pw,
            "w_qkv_c": wq,
            "w_bias_e": w_bias_e,
            "w_out_c": wo,
            "b_out_c": b_out[None, a * OUTC:(a + 1) * OUTC].astype(np.float32),
            "vecs": vecs,
            "bidx": bidx_np,
        })
    return in_maps


_NC_CACHE = None


def _get_nc():
    global _NC_CACHE
    if _NC_CACHE is None:
        _NC_CACHE = build_graph()
    return _NC_CACHE


def kernel(**inputs):
    from concourse.bass_utils import run_bass_kernel_spmd

    in_maps = prepare_in_maps(inputs)
    nc = _get_nc()
    res = run_bass_kernel_spmd(
        nc, in_maps, core_ids=list(range(NCORES)),
        trace=bool(int(os.environ.get("BASS_KERNEL_TRACE", "0"))),
        tmpdir=os.environ.get("BASS_KERNEL_TMPDIR"),
    )
    if res.exec_time_ns is not None:
        print(f"HW exec time: {res.exec_time_ns} ns", file=sys.stderr)

    out = np.zeros((B, N, D), np.float32)
    for c in range(NCORES):
        b, a = divmod(c, 4)
        out[b, :, a * OUTC:(a + 1) * OUTC] = res.results[c]["out_c"]
    return out


# revision 21
# speedup vs baseline: 1.2888x; 1.0399x over previous
"""Distributed Trainium2 Bass kernel for nn_Attention_57243324121446.

GQA attention (8 query groups, 1 kv head) with a pairwise-bias branch
(BatchRMSNorm -> exact gelu -> head projection, 4x nearest-neighbor upsample),
softclamp tanh, softmax, out-projection.

Sharding (8 cores): core c -> batch b = c//4, query groups {2*(c%4), 2*(c%4)+1}.
k/v are computed redundantly per core (single shared kv head). Pairwise is
sharded by (b, coarse-j block of 128 rows).

v2 layout (optimized):
 - Phase C (qkv+LN) overlaps phase B1 (pairwise stats streaming); the first
   16 of 32 pairwise tiles are cached in SBUF so B2 re-reads only half.
 - Bias exchange is an 8-rank AllToAll with duplicated head-pair chunks
   (wire ~0.9MB/half vs 3.7MB for the old 8-rank AllGather).
 - Attention is computed S^T = K^T q per j-chunk; P^T tiles feed AV matmuls
   as the *stationary* operand so the output lands as o[i, dv] with the
   softmax denominator accumulated for free in column 192 (ones column of v).
 - o is normalized per-i-partition (cheap [128,8] reciprocal), transposed via
   the PE into o^T, and AllGather'ed within the 4-core batch group per
   1024-token i-span; the out-projection for i-span 0 runs while span 1 is
   still computing.
"""

import os
import sys

sys.path.insert(0, "/opt/trn_rl_repo")

import numpy as np
import ml_dtypes

import concourse.bass as bass
import concourse.mybir as mybir
import concourse.tile as tile
from concourse.masks import make_identity


# --- workaround: this container's walrus caps CTRL instructions at 2 sem
# waits; Tile's kernel-tail drain can carry many. Split them across drains.
def _patched_drain_and_barrier(self, tick_clock, wait_clock):
    from concourse.vector_clock import ScopedClock
    drain_inst = self.nc.sync.drain()
    wait_clock.add_sem_waits(
        drain_inst.ins, ScopedClock({None: tick_clock.global_clock})
    )
    si = drain_inst.ins.sync_info
    if si is not None and len(si.on_wait) > 1:
        waits = list(si.on_wait)
        drain_inst.ins.sync_info = mybir.SyncInfo(
            on_wait=waits[:1], on_update=list(si.on_update)
        )
        for i in range(1, len(waits)):
            extra = self.nc.sync.drain()
            extra.ins.sync_info = mybir.SyncInfo(
                on_wait=waits[i:i + 1], on_update=[]
            )
    self.nc.all_engine_barrier()
    assert self.sems is not None
    popped = self.nc._tile_sem_poison_stack.pop()
    assert popped is self._sem_poison
    self.nc.clear_and_free_semaphores(list(self.sems.allocated().values()))
    self.nc.all_engine_barrier()


tile.TileContext._drain_and_barrier = _patched_drain_and_barrier


# --- workaround 2: this walrus accepts at most ONE sem wait per instruction.
# Rewrite the BIR json before compile: hoist excess waits onto same-engine
# Nop carriers inserted immediately before the offending instruction.
import json as _json
import concourse.bass_utils as _bass_utils
import concourse.bass2jax as _bass2jax


def _split_bir_multiwaits(bir_json):
    d = _json.loads(bir_json)
    mods = d.get("modules") or [d]
    for m in mods:
        for fn in m.get("functions", []):
            for bb in fn.get("blocks", []):
                out = []
                changed = False
                for ins in bb["instructions"]:
                    si = ins.get("sync_info")
                    w = (si or {}).get("on_wait") or []
                    if len(w) > 1 and ins.get("engine"):
                        eng = ins["engine"]
                        for i, wi in enumerate(w[:-1]):
                            out.append({
                                "debug": ins.get("debug"),
                                "engine": eng,
                                "ins": [{"dtype": "int32", "kind": "imm_value",
                                         "value": 0}],
                                "name": ins["name"] + f".sw{i}",
                                "opcode": "RegisterMove",
                                "outs": [{"dtype": "int32",
                                          "kind": "register_access",
                                          "regref": f"{eng}_zero"}],
                                "sync_info": {"on_update": [], "on_wait": [wi]},
                            })
                        si["on_wait"] = [w[-1]]
                        changed = True
                    out.append(ins)
                if changed:
                    bb["instructions"] = out
    return _json.dumps(d).encode()


_orig_compile_bir = _bass_utils.compile_bir_kernel


def _patched_compile_bir(bir_json, tmpdir, neff_name="file.neff"):
    return _orig_compile_bir(_split_bir_multiwaits(bir_json), tmpdir, neff_name)


_bass_utils.compile_bir_kernel = _patched_compile_bir
_bass2jax.compile_bir_kernel = _patched_compile_bir


# --- workaround 3: the agent image's antenv lacks axon_hooks, so the boot
# shim never registers the NTFF profile hook. Provide the module and install
# the ctypes hook ourselves so run_bass_kernel_spmd(trace=True) works.
def _install_ntff_hook():
    import types as _types
    mod = sys.modules.get("antenv.axon_hooks")
    if mod is None:
        mod = _types.ModuleType("antenv.axon_hooks")
        mod._hook = None
        def _set(h):
            mod._hook = h
        def _get():
            return mod._hook
        mod.set_axon_ntff_profile_hook = _set
        mod.get_axon_ntff_profile_hook = _get
        sys.modules["antenv.axon_hooks"] = mod
        import antenv as _antenv
        _antenv.axon_hooks = mod
    if mod._hook is None and os.path.exists("/opt/axon/libaxon_pjrt.so"):
        try:
            from trn_agent_boot.trn_boot import _ntff_profile_via_ctypes
            mod._hook = _ntff_profile_via_ctypes("/opt/axon/libaxon_pjrt.so")
        except Exception as e:
            print(f"ntff hook install failed: {e}", file=sys.stderr)


_install_ntff_hook()


BF16 = mybir.dt.bfloat16
FP16 = mybir.dt.float16
F32 = mybir.dt.float32
AF = mybir.ActivationFunctionType
ALU = mybir.AluOpType

B, N, D = 2, 2048, 1536
HEADS, KVH, DQK, DV = 8, 1, 128, 192
G = HEADS // KVH
NP, DP = 512, 128
SCALE = DQK ** -0.5
CLAMP = 5.0
MOMENTUM = 0.1
EPS = 1e-5

NCORES = 8
GPC = 2              # query groups per core
JBLK = NP // 4       # pairwise coarse-j rows per core = 128
ROWS = JBLK * NP     # pairwise rows per core = 65536
TOK = 128            # token chunk
NTOK = N // TOK      # 16
DCH = D // 128       # 12 d-model chunks
JC = N // 128        # 16 fine-j chunks
OUTC = D // 4        # 384 out cols per core
NPW = 32             # pairwise tiles of 2048 rows
KCACHE = 16          # pairwise tiles kept in SBUF between B1 and B2
MTOT = float(B * NP * NP)
HSP = N // 2         # i-span = 1024
J_ORDER = [j for q in range(4) for j in range(JC) if j % 4 == q]
RG8 = [list(range(NCORES))]
RG4 = [[0, 1, 2, 3], [4, 5, 6, 7]]


def _ap(base, dims):
    return bass.AP(tensor=base.tensor, offset=base.offset, ap=dims)


def build_graph():
    nc = bass.Bass()

    x_T = nc.declare_dram_parameter("x_T", [128, DCH, N], BF16, isOutput=False)
    pw_T = nc.declare_dram_parameter("pw_T", [128, ROWS], BF16, isOutput=False)
    w_qkv_c = nc.declare_dram_parameter("w_qkv_c", [128, DCH, 576], BF16, isOutput=False)
    w_bias_e = nc.declare_dram_parameter("w_bias_e", [128, 8], BF16, isOutput=False)
    w_out_c = nc.declare_dram_parameter("w_out_c", [128, DCH, OUTC], BF16, isOutput=False)
    b_out_c = nc.declare_dram_parameter("b_out_c", [1, OUTC], F32, isOutput=False)
    # vecs rows: 0 qw_eff,1 qb_eff,2 kw,3 kb,4 vw(192),5 vb(192),6 gamma,
    #            7 beta,8 rv9eps
    vecs = nc.declare_dram_parameter("vecs", [12, 192], F32, isOutput=False)
    bidx = nc.declare_dram_parameter("bidx", [GPC, JC, 128], mybir.dt.int32, isOutput=False)
    out_c = nc.declare_dram_parameter("out_c", [N, OUTC], F32, isOutput=True)

    with tile.TileContext(nc) as tc:
        with tc.tile_pool(name="const", bufs=1) as const, \
             tc.tile_pool(name="dram", bufs=1, space="DRAM") as dram:

            # ---------------- constants ----------------
            ident = const.tile([128, 128], BF16)
            make_identity(nc, ident[:])
            vec_sb = const.tile([128, 12], F32)
            nc.sync.dma_start(out=vec_sb[:], in_=_ap(vecs[:], [[1, 128], [192, 12]]))
            # vwb plane 0 = v_w broadcast, plane 1 = v_b broadcast (free dim)
            ones1 = const.tile([1, 128], BF16)
            nc.vector.memset(ones1[:], 1.0)
            # vwb_sb: col0 vw[0:128], col1 vb[0:128], col2 vw[128:192], col3 vb
            vwb_sb = const.tile([128, 4], F32)
            nc.sync.dma_start(
                out=vwb_sb[:, 0:2], in_=_ap(vecs[4, 0], [[1, 128], [192, 2]])
            )
            nc.sync.dma_start(
                out=vwb_sb[0:64, 2:4], in_=_ap(vecs[4, 128], [[1, 64], [192, 2]])
            )
            eps_sb = const.tile([128, 1], F32)
            nc.vector.memset(eps_sb[:], EPS)
            wq_sb = const.tile([128, DCH, 576], BF16)
            nc.sync.dma_start(out=wq_sb[:], in_=w_qkv_c[:])
            wb_sb = const.tile([128, 8], BF16)
            nc.sync.dma_start(out=wb_sb[:], in_=w_bias_e[:])
            bidx_sb = const.tile([128, GPC * JC], mybir.dt.int32)
            nc.sync.dma_start(
                out=bidx_sb[:], in_=_ap(bidx[:], [[1, 128], [128, GPC * JC]])
            )

            bias_gath = const.tile([128, GPC, JC, 512], BF16)
            qkT = const.tile([128, 3, N], BF16)       # q0^T, q1^T, k^T
            vfull = const.tile([128, NTOK, 208], BF16)  # v~*vw cols 0:192, 192=1
            nc.vector.memset(vfull[:], 0.0)
            nc.vector.memset(vfull[:, :, 192:193], 1.0)
            stats = const.tile([128, 8, 16, 6], F32)
            mv = const.tile([128, 2], F32)
            part = const.tile([128, 2], F32)
            gsum = const.tile([128, 2], F32)
            scl = const.tile([128, 2], F32)
            wo_sb = const.tile([128, DCH, OUTC], BF16)
            bout_bc = const.tile([128, OUTC], F32)

            var_in = dram.tile([128, 2], F32)
            var_out = dram.tile([128, 2], F32, addr_space="Shared")
            a2a_in_q = [dram.tile([512, 512], BF16, name=f"a2ai{q}")
                        for q in range(4)]
            a2a_out_q = [dram.tile([512, 512], BF16, name=f"a2ao{q}")
                         for q in range(4)]
            SPANS = [(0, 1024), (1024, 512), (1536, 512)]
            ot_own = [[dram.tile([DV, ln], BF16, name=f"oto{i}{g}")
                       for g in range(2)] for i, (o, ln) in enumerate(SPANS)]
            ot_all = [[dram.tile([4 * DV, ln], BF16, name=f"ota{i}{g}")
                       for g in range(2)] for i, (o, ln) in enumerate(SPANS)]

            # ---------------- PE warmup (HAM unthrottle) ----------------
            with tc.tile_pool(name="warm", bufs=1) as warm, \
                 tc.tile_pool(name="warmps", bufs=1, space="PSUM") as warmps:
                wps = warmps.tile([128, 512], F32)
                wsb = warm.tile([128, 128], F32)
                wdr = dram.tile([128, 128], F32)
                for r in range(100):
                    nc.tensor.matmul(
                        wps[:, 0:128], ident[:], ident[:],
                        start=True, stop=True,
                    )
                nc.vector.tensor_copy(wsb[:, 0:128], wps[:, 0:128])
                nc.sync.dma_start(out=wdr[:], in_=wsb[:])

            # ---------------- B1 (pairwise stats) + C (qkv/LN), overlapped ----------------
            with tc.tile_pool(name="cpool", bufs=1) as cpool, \
                 tc.tile_pool(name="cps", bufs=2, space="PSUM") as cps:
                xt_sb = cpool.tile([128, DCH, N], BF16, name="xt_sb")
                nc.sync.dma_start(out=xt_sb[:], in_=x_T[:])
                qkv_sb = cpool.tile([128, NTOK, 576], BF16, name="qkv_sb")
                st_all = cpool.tile([128, NTOK, 4, 6], F32, name="st_all")
                mv_all = cpool.tile([128, NTOK, 4, 2], F32, name="mv_all")
                std_all = cpool.tile([128, NTOK * 4], F32, name="std_all")
                pwt = [None] * 8
                for tb in range(8):
                    pwt[tb] = cpool.tile([128, 4, 2048], BF16, tag="pw", bufs=3,
                                         name="pwt")
                    nc.sync.dma_start(
                        out=pwt[tb][:], in_=pw_T[:, tb * 8192:(tb + 1) * 8192]
                    )
                for t in range(NTOK):
                    ps_qkv = cps.tile([128, 576], F32, tag="qkv")
                    for c in range(DCH):
                        nc.tensor.matmul(
                            ps_qkv[:, 0:512], xt_sb[:, c, t * TOK:(t + 1) * TOK],
                            wq_sb[:, c, 0:512], start=(c == 0),
                            stop=(c == DCH - 1),
                        )
                        nc.tensor.matmul(
                            ps_qkv[:, 512:576], xt_sb[:, c, t * TOK:(t + 1) * TOK],
                            wq_sb[:, c, 512:576], start=(c == 0),
                            stop=(c == DCH - 1),
                        )
                    nc.scalar.copy(qkv_sb[:, t, :], ps_qkv[:])
                    # interleave B1 stats: 8 per C tile (pw big-tile t//2)
                    tb, hq = t // 2, (t % 2) * 8
                    for q2 in range(hq, hq + 8):
                        nc.vector.bn_stats(
                            out=stats[:, tb, q2, :],
                            in_=pwt[tb][:, q2 // 4, (q2 % 4) * 512:(q2 % 4 + 1) * 512],
                        )
                # aggregate + AllReduce as soon as stats are done
                nc.vector.bn_aggr(
                    out=mv[:], in_=stats[:].rearrange("p a b c -> p (a b) c")
                )
                nc.vector.tensor_scalar_mul(part[:, 0:1], mv[:, 0:1], float(ROWS))
                nc.vector.tensor_tensor(
                    out=part[:, 1:2], in0=mv[:, 0:1], in1=mv[:, 0:1], op=ALU.mult
                )
                nc.vector.tensor_tensor(
                    out=part[:, 1:2], in0=part[:, 1:2], in1=mv[:, 1:2], op=ALU.add
                )
                nc.vector.tensor_scalar_mul(part[:, 1:2], part[:, 1:2], float(ROWS))
                nc.sync.dma_start(out=var_in[:], in_=part[:])
                nc.gpsimd.collective_compute(
                    "AllReduce", ALU.add,
                    replica_groups=RG8,
                    ins=[var_in[:].opt()], outs=[var_out[:].opt()],
                )
                nc.sync.dma_start(out=gsum[:], in_=var_out[:])

                # C LN stats (after the AllReduce trigger in the DVE queue)
                for t in range(NTOK):
                    for sr in range(4):
                        lo, hi = (sr * 128, (sr + 1) * 128) if sr < 3 else (384, 576)
                        nc.vector.bn_stats(
                            out=st_all[:, t, sr, :], in_=qkv_sb[:, t, lo:hi]
                        )
                        nc.vector.bn_aggr(
                            out=mv_all[:, t, sr, :], in_=st_all[:, t, sr, :]
                        )
                # bulk rsqrt for all 64 (tile, subrange) pairs
                nc.scalar.activation(
                    out=std_all[:],
                    in_=mv_all[:].rearrange("p t s d -> p (t s d)")[:, 1::2],
                    func=AF.Sqrt, bias=eps_sb[:],
                )
                nc.vector.reciprocal(out=std_all[:], in_=std_all[:])
                nmur = cpool.tile([128, NTOK * 4], F32, name="nmur")
                nc.vector.tensor_tensor(
                    out=nmur[:],
                    in0=mv_all[:].rearrange("p t s d -> p (t s d)")[:, 0::2],
                    in1=std_all[:], op=ALU.mult,
                )
                nc.vector.tensor_scalar_mul(nmur[:], nmur[:], -1.0)
                for t in range(NTOK):
                    nrm = cpool.tile([128, 576], BF16, tag="nrm", bufs=3)
                    for sr in range(4):
                        lo, hi = (sr * 128, (sr + 1) * 128) if sr < 3 else (384, 576)
                        nc.vector.tensor_scalar(
                            out=nrm[:, lo:hi], in0=qkv_sb[:, t, lo:hi],
                            scalar1=std_all[:, 4 * t + sr:4 * t + sr + 1],
                            scalar2=nmur[:, 4 * t + sr:4 * t + sr + 1],
                            op0=ALU.mult, op1=ALU.add,
                        )
                    nc.vector.tensor_copy(vfull[:, t, 0:192], nrm[:, 384:576])
                    for sr in range(3):
                        ps_tr = cps.tile([128, 128], BF16, tag="tr")
                        nc.tensor.transpose(
                            ps_tr[:], nrm[:, sr * 128:(sr + 1) * 128], ident[:]
                        )
                        av = 0 if sr < 2 else 2
                        nc.scalar.activation(
                            out=qkT[:, sr, t * TOK:(t + 1) * TOK], in_=ps_tr[:],
                            func=AF.Identity, scale=vec_sb[:, av:av + 1],
                            bias=vec_sb[:, av + 1:av + 2],
                        )

                # ---------------- B-scale ----------------
                nc.vector.tensor_scalar_mul(scl[:, 0:1], gsum[:, 0:1], 1.0 / MTOT)
                nc.vector.tensor_tensor(
                    out=scl[:, 0:1], in0=scl[:, 0:1], in1=scl[:, 0:1], op=ALU.mult
                )
                nc.vector.tensor_scalar_mul(scl[:, 1:2], gsum[:, 1:2], 1.0 / MTOT)
                nc.vector.tensor_tensor(
                    out=scl[:, 0:1], in0=scl[:, 1:2], in1=scl[:, 0:1],
                    op=ALU.subtract
                )
                nc.vector.tensor_scalar(
                    out=scl[:, 0:1], in0=scl[:, 0:1], scalar1=MOMENTUM,
                    scalar2=vec_sb[:, 8:9], op0=ALU.mult, op1=ALU.add,
                )
                nc.scalar.activation(out=scl[:, 0:1], in_=scl[:, 0:1], func=AF.Sqrt)
                nc.vector.reciprocal(out=scl[:, 0:1], in_=scl[:, 0:1])
                nc.vector.tensor_tensor(
                    out=scl[:, 0:1], in0=scl[:, 0:1], in1=vec_sb[:, 6:7],
                    op=ALU.mult
                )
                nc.vector.tensor_copy(scl[:, 1:2], vec_sb[:, 7:8])

            # ---------------- B2: gelu + bias projection ----------------
            # A2A in-buffer rows: dup*512 + h*64 + jl_half (jl_half < 64)
            with tc.tile_pool(name="b2", bufs=1) as b2, \
                 tc.tile_pool(name="b2ps", bufs=2, space="PSUM") as b2ps:
                # warm burst gated on scl: re-arms the PE clock right before
                # the projection matmuls start
                wmd = b2.tile([128, 128], BF16, name="wmd")
                nc.vector.tensor_scalar(
                    out=wmd[:], in0=ident[:], scalar1=scl[:, 0:1], scalar2=None,
                    op0=ALU.mult,
                )
                wps2 = b2ps.tile([128, 512], F32, tag="psb", name="wps2")
                for r in range(60):
                    nc.tensor.matmul(
                        wps2[:, 0:128], wmd[:], ident[:], start=True, stop=True
                    )
                for hb in range(8):
                    pt2 = b2.tile([128, 4, 2048], BF16, tag="pw2", bufs=2)
                    nc.sync.dma_start(
                        out=pt2[:], in_=pw_T[:, hb * 8192:(hb + 1) * 8192]
                    )
                    acc = b2.tile([128, 4, 512], BF16, tag="acc", bufs=2)
                    for tl in range(4):
                        gel = b2.tile([128, 2048], BF16, tag="gel", bufs=3)
                        nc.scalar.activation(
                            out=gel[:], in_=pt2[:, tl, :], func=AF.Gelu,
                            bias=scl[:, 1:2], scale=scl[:, 0:1],
                        )
                        ps_b = b2ps.tile([128, 512], F32, tag="psb")
                        # full-tile init: the copy below reads all 128
                        # partitions but only rows 32q..32q+8 are written
                        nc.vector.memset(ps_b[:], 0.0)
                        for q in range(4):
                            nc.tensor.matmul(
                                ps_b[32 * q:32 * q + 8, :], wb_sb[:],
                                gel[:, q * 512:(q + 1) * 512],
                                start=True, stop=True,
                                tile_position=(0, 32 * q),
                            )
                        nc.vector.tensor_copy(acc[:, tl, :], ps_b[:])
                    dst = a2a_in_q[hb // 2]
                    for q in range(4):
                        nc.sync.dma_start(
                            out=_ap(
                                dst[(hb % 2) * 16 + q, 0],
                                [[32 * 512, 8], [4 * 512, 4], [1, 512]],
                            ),
                            in_=acc[32 * q:32 * q + 8, :, :],
                        )
                    if hb % 2 == 1:
                        qq = hb // 2
                        nc.sync.dma_start(
                            out=a2a_in_q[qq][256:512, :],
                            in_=a2a_in_q[qq][0:256, :]
                        )
                        nc.gpsimd.collective_compute(
                            "AllToAll", ALU.bypass,
                            replica_groups=RG8,
                            ins=[a2a_in_q[qq][:].opt()],
                            outs=[a2a_out_q[qq][:].opt()],
                        )

            # load E-phase constants (issued late so they don't delay pw/x)
            nc.sync.dma_start(out=wo_sb[:], in_=w_out_c[:])
            nc.sync.dma_start(
                out=bout_bc[:], in_=_ap(b_out_c[:], [[0, 128], [1, OUTC]])
            )

            # ---------------- D: attention, E: out projection ----------------
            with tc.tile_pool(name="dper", bufs=1) as dper, \
                 tc.tile_pool(name="dsb", bufs=2) as dsb, \
                 tc.tile_pool(name="osb", bufs=2) as osb, \
                 tc.tile_pool(name="dps", bufs=2, space="PSUM") as dps, \
                 tc.tile_pool(name="dpo", bufs=1, space="PSUM") as dpo:
                for j in J_ORDER:
                    for g in range(GPC):
                        src_t = a2a_out_q[j % 4]
                        nc.gpsimd.indirect_dma_start(
                            out=bias_gath[:, g, j, :],
                            out_offset=None,
                            in_=src_t[:],
                            in_offset=bass.IndirectOffsetOnAxis(
                                ap=bidx_sb[:, g * JC + j:g * JC + j + 1], axis=0
                            ),
                        )

                pending = []   # deferred post-processing closures

                def flush_pending():
                    for f in pending:
                        f()
                    pending.clear()

                def d_pass(sp, g, mid_cb=None):
                    i0, ln = SPANS[sp]
                    nh = ln // 512
                    ps = {}
                    stash = []

                    def emit_avs(j, pT, first, last):
                        for h2 in range(nh):
                            nc.tensor.matmul(
                                ps["oa"][:, h2 * 512:(h2 + 1) * 512],
                                vfull[:, j, 0:128],
                                pT[:, h2 * 512:(h2 + 1) * 512],
                                start=first, stop=last,
                            )
                        for h2 in range(nh):
                            nc.tensor.matmul(
                                ps["ob"][:, h2 * 512:(h2 + 1) * 512],
                                vfull[:, j, 128:193],
                                pT[:, h2 * 512:(h2 + 1) * 512],
                                start=first, stop=last,
                            )

                    for jj, j in enumerate(J_ORDER):
                        ps_s = dps.tile([128, 1024], F32, tag="s", name="ps_s")
                        for h2 in range(nh):
                            nc.tensor.matmul(
                                ps_s[:, h2 * 512:(h2 + 1) * 512],
                                qkT[:, 2, j * 128:(j + 1) * 128],
                                qkT[:, g, i0 + h2 * 512:i0 + (h2 + 1) * 512],
                                start=True, stop=True,
                            )
                        bt = bias_gath[:, g, j, i0 // 4:i0 // 4 + ln // 4]
                        bb = _ap(bt, [bt.ap[0], bt.ap[1], [0, 4]])
                        u = dsb.tile([128, 1024], FP16, tag="u", name="u")
                        nc.vector.tensor_tensor(
                            out=u[:, 0:ln].rearrange("p (a b) -> p a b", b=4),
                            in0=ps_s[:, 0:ln].rearrange("p (a b) -> p a b", b=4),
                            in1=bb, op=ALU.add,
                        )
                        ut = dsb.tile([128, 1024], FP16, tag="ut", name="ut")
                        nc.scalar.activation(
                            out=ut[:, 0:ln], in_=u[:, 0:ln], func=AF.Tanh
                        )
                        pT = dsb.tile([128, 1024], BF16, tag="pT", bufs=10,
                                      name="pT")
                        nc.scalar.activation(
                            out=pT[:, 0:ln], in_=ut[:, 0:ln], func=AF.Exp,
                            scale=CLAMP
                        )
                        if jj < 8:
                            stash.append((j, pT))
                        else:
                            if jj == 8:
                                flush_pending()
                                ps["oa"] = dpo.tile([128, 1024], F32, tag="oa",
                                                    name="ps_oa")
                                ps["ob"] = dpo.tile([65, 1024], F32, tag="ob",
                                                    name="ps_ob")
                                if mid_cb is not None:
                                    mid_cb()
                                for k2, (j0, pT0) in enumerate(stash):
                                    emit_avs(j0, pT0, k2 == 0, False)
                            emit_avs(j, pT, False, jj == JC - 1)

                    def post(sp=sp, g=g, ps_oa=ps["oa"], ps_ob=ps["ob"],
                             ln=ln, nh=nh):
                        rd = osb.tile([1, 1024], F32, tag="rd", name="rd")
                        nc.vector.reciprocal(rd[:, 0:ln], ps_ob[64:65, 0:ln])
                        rdb = osb.tile([1, 1024], BF16, tag="rdb", name="rdb")
                        nc.vector.tensor_copy(rdb[:, 0:ln], rd[:, 0:ln])
                        ps_bc = dps.tile([128, 1024], F32, tag="s", name="ps_bc")
                        for h2 in range(nh):
                            nc.tensor.matmul(
                                ps_bc[:, h2 * 512:(h2 + 1) * 512], ones1[:],
                                rdb[:, h2 * 512:(h2 + 1) * 512],
                                start=True, stop=True,
                            )
                        bc_sb = osb.tile([128, 1024], BF16, tag="bc_sb",
                                         name="bc_sb")
                        nc.vector.tensor_copy(bc_sb[:, 0:ln], ps_bc[:, 0:ln])
                        oa = osb.tile([128, 1024], BF16, tag="oa_sb", name="oa")
                        ob = osb.tile([64, 1024], BF16, tag="ob_sb", name="ob")
                        nc.vector.scalar_tensor_tensor(
                            out=oa[:, 0:ln], in0=bc_sb[:, 0:ln],
                            scalar=vwb_sb[:, 0:1], in1=ps_oa[:, 0:ln],
                            op0=ALU.mult, op1=ALU.mult,
                        )
                        nc.vector.tensor_scalar(
                            out=oa[:, 0:ln], in0=oa[:, 0:ln], scalar1=1.0,
                            scalar2=vwb_sb[:, 1:2], op0=ALU.mult, op1=ALU.add,
                        )
                        nc.vector.scalar_tensor_tensor(
                            out=ob[:, 0:ln], in0=bc_sb[0:64, 0:ln],
                            scalar=vwb_sb[0:64, 2:3], in1=ps_ob[0:64, 0:ln],
                            op0=ALU.mult, op1=ALU.mult,
                        )
                        nc.vector.tensor_scalar(
                            out=ob[:, 0:ln], in0=ob[:, 0:ln], scalar1=1.0,
                            scalar2=vwb_sb[0:64, 3:4], op0=ALU.mult, op1=ALU.add,
                        )
                        nc.sync.dma_start(
                            out=ot_own[sp][g][0:128, :], in_=oa[:, 0:ln]
                        )
                        nc.sync.dma_start(
                            out=ot_own[sp][g][128:DV, :], in_=ob[:, 0:ln]
                        )
                        nc.gpsimd.collective_compute(
                            "AllGather", ALU.bypass,
                            replica_groups=RG4,
                            ins=[ot_own[sp][g][:].opt()],
                            outs=[ot_all[sp][g][:].opt()],
                        )

                    pending.append(post)

                mge_cache = {}

                def e_pass(sp, tls):
                    i0, ln = SPANS[sp]
                    mge = mge_cache.get(sp)
                    if mge is None:
                        mge = dper.tile([128, DCH, ln], BF16, name=f"mge{sp}",
                                        tag=f"mge{sp}")
                        mge_cache[sp] = mge
                    # merged^T row r = kc*128+p = h*192+dv; head h lives in
                    # ot_all[sp][h%2] rows (h//2)*192+dv
                    for h in (range(8) if tls[0] == 0 else []):
                        src = ot_all[sp][h % 2]
                        r0 = h * DV
                        a0 = (h // 2) * DV
                        cuts = [r0, ((r0 + 127) // 128) * 128, r0 + DV]
                        if cuts[1] == cuts[0]:
                            cuts = [r0, r0 + 128, r0 + DV]
                        for ci in range(len(cuts) - 1):
                            lo, hi = cuts[ci], cuts[ci + 1]
                            nc.sync.dma_start(
                                out=mge[lo % 128:(lo % 128) + (hi - lo),
                                        lo // 128, :],
                                in_=src[a0 + lo - r0:a0 + hi - r0, :],
                            )
                    for tl in tls:
                        t = i0 // TOK + tl
                        ps_o = dpo.tile(
                            [128, OUTC], F32,
                            tag=("oa" if tl % 2 == 0 else "ob"), name="ps_o",
                        )
                        for kc in range(DCH):
                            nc.tensor.matmul(
                                ps_o[:], mge[:, kc, tl * 128:(tl + 1) * 128],
                                wo_sb[:, kc, :], start=(kc == 0),
                                stop=(kc == DCH - 1),
                            )
                        o_out = osb.tile([128, OUTC], F32, tag="oout",
                                         name="o_out")
                        nc.vector.tensor_tensor(
                            out=o_out[:], in0=ps_o[:], in1=bout_bc[:],
                            op=ALU.add
                        )
                        nc.sync.dma_start(
                            out=out_c[t * TOK:(t + 1) * TOK, :], in_=o_out[:]
                        )

                d_pass(0, 0)
                d_pass(0, 1)
                d_pass(1, 0)
                d_pass(1, 1, mid_cb=lambda: e_pass(0, range(0, 4)))
                d_pass(2, 0, mid_cb=lambda: e_pass(0, range(4, 8)))
                d_pass(2, 1, mid_cb=lambda: e_pass(1, range(4)))
                flush_pending()
                e_pass(2, range(4))


    return nc


def prepare_in_maps(inputs):
    bf16 = ml_dtypes.bfloat16
    x = np.asarray(inputs["x"], np.float32)
    pairwise = np.asarray(inputs["pairwise"], np.float32)
    w_qkv = np.asarray(inputs["w_qkv"], np.float32)
    q_w = np.asarray(inputs["q_w"], np.float32)
    q_b = np.asarray(inputs["q_b"], np.float32)
    k_w = np.asarray(inputs["k_w"], np.float32)
    k_b = np.asarray(inputs["k_b"], np.float32)
    v_w = np.asarray(inputs["v_w"], np.float32)
    v_b = np.asarray(inputs["v_b"], np.float32)
    gamma = np.asarray(inputs["bias_gamma"], np.float32)
    beta = np.asarray(inputs["bias_beta"], np.float32)
    rvar = np.asarray(inputs["bias_running_var"], np.float32)
    w_bias = np.asarray(inputs["w_bias"], np.float32)
    w_out = np.asarray(inputs["w_out"], np.float32)
    b_out = np.asarray(inputs["b_out"], np.float32)

    vecs = np.zeros((12, 192), np.float32)
    vecs[0, :128] = q_w * (SCALE / CLAMP)
    vecs[1, :128] = q_b * (SCALE / CLAMP)
    vecs[2, :128] = k_w
    vecs[3, :128] = k_b
    vecs[4, :192] = v_w
    vecs[5, :192] = v_b
    vecs[6, :128] = gamma
    vecs[7, :128] = beta
    vecs[8, :128] = (1.0 - MOMENTUM) * rvar + EPS

    w_bias_e = (w_bias / CLAMP).astype(bf16)

    in_maps = []
    for c in range(NCORES):
        b, a = divmod(c, 4)
        xt = np.ascontiguousarray(
            x[b].T.reshape(DCH, 128, N).transpose(1, 0, 2)
        ).astype(bf16)
        pw = pairwise[b, :, a * JBLK:(a + 1) * JBLK, :]        # [i, jl, dp]
        pw = np.ascontiguousarray(pw.transpose(2, 1, 0).reshape(128, ROWS)
                                  ).astype(bf16)
        qcols = w_qkv[:, 2 * a * DQK:(2 * a + 2) * DQK]
        kcols = w_qkv[:, G * DQK:G * DQK + DQK]
        vcols = w_qkv[:, G * DQK + DQK:]
        wq = np.concatenate([qcols, kcols, vcols], axis=1)     # [1536, 576]
        wq = np.ascontiguousarray(
            wq.reshape(DCH, 128, 576).transpose(1, 0, 2)).astype(bf16)
        wo = w_out[:, a * OUTC:(a + 1) * OUTC]
        wo = np.ascontiguousarray(
            wo.reshape(DCH, 128, OUTC).transpose(1, 0, 2)).astype(bf16)
        # bias gather rows in the A2A out quarter-buffer [512, 512]:
        # row = (b*4 + block)*64 + g*32 + p//4
        gg, jj, pp = np.meshgrid(
            np.arange(GPC), np.arange(JC), np.arange(128), indexing="ij"
        )
        bidx_np = (
            (b * 4 + jj // 4) * 64 + gg * 32 + pp // 4
        ).astype(np.int32)
        in_maps.append({
            "x_T": xt,
            "pw_T": pw,
            "w_qkv_c": wq,
            "w_bias_e": w_bias_e,
            "w_out_c": wo,
            "b_out_c": b_out[None, a * OUTC:(a + 1) * OUTC].astype(np.float32),
            "vecs": vecs,
            "bidx": bidx_np,
        })
    return in_maps


_NC_CACHE = None


def _get_nc():
    global _NC_CACHE
    if _NC_CACHE is None:
        _NC_CACHE = build_graph()
    return _NC_CACHE


def kernel(**inputs):
    from concourse.bass_utils import run_bass_kernel_spmd

    in_maps = prepare_in_maps(inputs)
    nc = _get_nc()
    res = run_bass_kernel_spmd(
        nc, in_maps, core_ids=list(range(NCORES)),
        trace=bool(int(os.environ.get("BASS_KERNEL_TRACE", "0"))),
        tmpdir=os.environ.get("BASS_KERNEL_TMPDIR"),
    )
    if res.exec_time_ns is not None:
        print(f"HW exec time: {res.exec_time_ns} ns", file=sys.stderr)

    out = np.zeros((B, N, D), np.float32)
    for c in range(NCORES):
        b, a = divmod(c, 4)
        out[b, :, a * OUTC:(a + 1) * OUTC] = res.results[c]["out_c"]
    return out
